# revision 1
# baseline (speedup 1.0000x reference)
"""BiLSTM-CRF loss on 8 Trainium2 NeuronCores.

Strategy:
  - Direction-split: cores 0-3 run the forward LSTM, cores 4-7 the backward
    LSTM (on host-pre-flipped input). Within each group the batch (32) is
    sharded 4 ways -> 8 sequences per core.
  - Device kernel A: input projections x @ W_ih.T + (b_ih+b_hh) as one big
    matmul per core (bias folded in via a ones-row matmul).
  - Device kernel B: 64 unrolled LSTM recurrence steps (compiled once, called
    8x with c/hT state roundtrip). Recurrent matmul is lhsT=h.T (tiny
    stationary), rhs=W_hh.T resident in SBUF; x-projection is folded into the
    same PSUM accumulation group via an identity-stationary matmul.
  - Host (numpy): embedding gather, sequence flips, emissions, CRF
    forward/gold score (cheap, O(T*B*L^2)).
"""
import sys
import numpy as np

sys.path.insert(0, '/opt/trn_rl_repo')

import concourse.bacc as bacc
import concourse.mybir as mybir
from concourse.tile import TileContext
from concourse.bass_utils import run_bass_kernel_spmd
import ml_dtypes

BF16 = ml_dtypes.bfloat16
F32 = np.float32

B, T = 32, 512
V, D, L = 50257, 512, 48
G = 4 * D  # 2048 gate width
NCORES = 8
BL = 8       # sequences per core (dir-split: 4 cores x 8 = 32 per direction)
CH = 128     # recurrence steps per kernel-B invocation
NCH = T // CH
NTOK = T * BL  # tokens per core = 4096
MT = NTOK // 128  # M-tiles in projection = 32

_SIG = mybir.ActivationFunctionType.Sigmoid
_TANH = mybir.ActivationFunctionType.Tanh

_cache = {}


def _build_proj():
    nc = bacc.Bacc()
    dt = mybir.dt
    embT = nc.declare_dram_parameter("embT", [128, 4 * NTOK], dt.bfloat16, isOutput=False)
    wih = nc.declare_dram_parameter("wih", [128, 4 * G], dt.bfloat16, isOutput=False)
    bias = nc.declare_dram_parameter("bias", [1, G], dt.bfloat16, isOutput=False)
    ones = nc.declare_dram_parameter("ones", [1, 128], dt.bfloat16, isOutput=False)
    xp = nc.declare_dram_parameter("xp", [MT, 128, G], dt.bfloat16, isOutput=True)

    with TileContext(nc) as tc:
        with (
            tc.tile_pool(name="const", bufs=1) as cpool,
            tc.tile_pool(name="psum", bufs=2, space="PSUM") as ppool,
            tc.tile_pool(name="out", bufs=3) as opool,
        ):
            embT_sb = cpool.tile([128, 4 * NTOK], dt.bfloat16)
            wih_sb = cpool.tile([128, 4 * G], dt.bfloat16)
            bias_sb = cpool.tile([1, G], dt.bfloat16)
            ones_sb = cpool.tile([1, 128], dt.bfloat16)
            nc.sync.dma_start(out=embT_sb[:], in_=embT[:])
            nc.sync.dma_start(out=wih_sb[:], in_=wih[:])
            nc.sync.dma_start(out=bias_sb[:], in_=bias[:])
            nc.sync.dma_start(out=ones_sb[:], in_=ones[:])
            for m in range(MT):
                ps = ppool.tile([128, G], dt.float32)
                for nb in range(4):
                    o = ps[:, nb * 512:(nb + 1) * 512]
                    for kc in range(4):
                        nc.tensor.matmul(
                            o,
                            embT_sb[:, kc * NTOK + m * 128: kc * NTOK + (m + 1) * 128],
                            wih_sb[:, kc * G + nb * 512: kc * G + (nb + 1) * 512],
                            start=(kc == 0), stop=False)
                    nc.tensor.matmul(
                        o, ones_sb[0:1, :], bias_sb[0:1, nb * 512:(nb + 1) * 512],
                        start=False, stop=True)
                ot = opool.tile([128, G], dt.bfloat16)
                nc.vector.tensor_copy(ot[:], ps[:])
                nc.sync.dma_start(out=xp[m], in_=ot[:])
    nc.finalize()
    return nc


def _build_rec():
    nc = bacc.Bacc()
    dt = mybir.dt
    xpc = nc.declare_dram_parameter("xpc", [CH, 4, BL, 512], dt.bfloat16, isOutput=False)
    whh = nc.declare_dram_parameter("whh", [128, 4 * G], dt.bfloat16, isOutput=False)
    i8 = nc.declare_dram_parameter("i8", [128, 8], dt.bfloat16, isOutput=False)
    c_in = nc.declare_dram_parameter("c_in", [BL, D], dt.float32, isOutput=False)
    hT_in = nc.declare_dram_parameter("hT_in", [128, 4 * BL], dt.bfloat16, isOutput=False)
    hs = nc.declare_dram_parameter("hs", [CH, BL, D], dt.bfloat16, isOutput=True)
    c_out = nc.declare_dram_parameter("c_out", [BL, D], dt.float32, isOutput=True)
    hT_out = nc.declare_dram_parameter("hT_out", [128, 4 * BL], dt.bfloat16, isOutput=True)

    with TileContext(nc) as tc:
        with (
            tc.tile_pool(name="const", bufs=1) as cpool,
            tc.tile_pool(name="xp", bufs=3) as xpool,
            tc.tile_pool(name="state", bufs=2) as spool,
            tc.tile_pool(name="gates", bufs=2) as gpool,
            tc.tile_pool(name="h", bufs=3) as hpool,
            tc.tile_pool(name="pg", bufs=1, space="PSUM") as pgpool,
            tc.tile_pool(name="pt", bufs=2, space="PSUM") as ptpool,
        ):
            whh_sb = cpool.tile([128, 4 * G], dt.bfloat16)
            i8_sb = cpool.tile([128, 8], dt.bfloat16)
            nc.sync.dma_start(out=whh_sb[:], in_=whh[:])
            nc.sync.dma_start(out=i8_sb[:], in_=i8[:])
            c_prev = spool.tile([BL, D], dt.float32, tag="c")
            nc.sync.dma_start(out=c_prev[:], in_=c_in[:])
            hT_prev = spool.tile([128, 4 * BL], dt.bfloat16, tag="hT")
            nc.sync.dma_start(out=hT_prev[:], in_=hT_in[:])

            for j in range(CH):
                xp_sb = xpool.tile([128, 512], dt.bfloat16, tag="xp")
                for nb in range(4):
                    nc.sync.dma_start(out=xp_sb[32 * nb:32 * nb + BL, :],
                                      in_=xpc[j, nb])
                # per-bank PSUM tiles: gate activations start as soon as
                # their own bank's accumulation group finishes
                pgs = [pgpool.tile([BL, 512], dt.float32, tag=f"pg{nb}",
                                   name=f"pg{nb}") for nb in range(4)]
                for nb in range(4):
                    nc.tensor.matmul(
                        pgs[nb][:], i8_sb[32 * nb:32 * nb + BL, :],
                        xp_sb[32 * nb:32 * nb + BL, :], start=True, stop=False,
                        tile_position=(32 * nb, 0))
                acts = []
                for nb in range(4):
                    for kc in range(4):
                        nc.tensor.matmul(
                            pgs[nb][:], hT_prev[:, kc * BL:(kc + 1) * BL],
                            whh_sb[:, kc * G + nb * 512: kc * G + (nb + 1) * 512],
                            start=False, stop=(kc == 3))
                    a_sb = gpool.tile([BL, D], dt.bfloat16, tag=f"act{nb}",
                                      name=f"act{nb}")
                    nc.scalar.activation(a_sb[:], pgs[nb][:],
                                         _TANH if nb == 2 else _SIG)
                    acts.append(a_sb)
                i_sb, f_sb, g_sb, o_sb = acts
                ig = gpool.tile([BL, D], dt.float32, tag="ig")
                nc.vector.tensor_mul(ig[:], i_sb[:], g_sb[:])
                fc = gpool.tile([BL, D], dt.float32, tag="fc")
                nc.vector.tensor_mul(fc[:], f_sb[:], c_prev[:])
                c_new = spool.tile([BL, D], dt.float32, tag="c")
                nc.vector.tensor_add(c_new[:], ig[:], fc[:])
                tc_sb = gpool.tile([BL, D], dt.bfloat16, tag="tc")
                nc.scalar.activation(tc_sb[:], c_new[:], _TANH)
                h_sb = hpool.tile([BL, D], dt.bfloat16, tag="h")
                nc.vector.tensor_mul(h_sb[:], o_sb[:], tc_sb[:])
                nc.sync.dma_start(out=hs[j], in_=h_sb[:])
                pt = ptpool.tile([128, 4 * BL], dt.bfloat16, tag="pt")
                for kc in range(4):
                    nc.tensor.transpose(
                        pt[:, kc * BL:(kc + 1) * BL],
                        h_sb[:, kc * 128:(kc + 1) * 128], i8_sb[0:8, :])
                hT_new = spool.tile([128, 4 * BL], dt.bfloat16, tag="hT")
                nc.vector.tensor_copy(hT_new[:], pt[:])
                c_prev, hT_prev = c_new, hT_new
            nc.sync.dma_start(out=c_out[:], in_=c_prev[:])
            nc.sync.dma_start(out=hT_out[:], in_=hT_prev[:])
    nc.finalize()
    return nc


def _chunk128(a):
    """[512, N] -> [128, 4*N] with k-chunk kc at cols [kc*N:(kc+1)*N]."""
    n = a.shape[1]
    return np.ascontiguousarray(
        a.reshape(4, 128, n).transpose(1, 0, 2).reshape(128, 4 * n))


def _seq_flip(x, lengths):
    t = np.arange(x.shape[1])[None, :]
    idx = lengths[:, None] - 1 - t
    idx = np.where(idx >= 0, idx, t)
    return np.take_along_axis(x, idx[:, :, None], axis=1)


def _logsumexp(a, axis):
    m = np.max(a, axis=axis, keepdims=True)
    return np.squeeze(m, axis) + np.log(np.sum(np.exp(a - m), axis=axis))


def kernel(tokens, tags, lengths, embed, W_ih_f, W_hh_f, b_ih_f, b_hh_f,
           W_ih_b, W_hh_b, b_ih_b, b_hh_b, init_hidden, W_emit, b_emit,
           start_trans, trans, end_trans):
    tokens = np.asarray(tokens).astype(np.int64)
    tags = np.asarray(tags).astype(np.int64)
    lengths = np.asarray(lengths).astype(np.int64)
    embed = np.asarray(embed, F32)

    if "proj" not in _cache:
        _cache["proj"] = _build_proj()
        _cache["rec"] = _build_rec()
    nc_p, nc_r = _cache["proj"], _cache["rec"]

    emb = embed[tokens]                      # [B,T,D] f32
    embr = _seq_flip(emb, lengths)           # reversed input for bwd lstm

    # ---- per-core packing ----
    ones = np.ones((1, 128), BF16)
    i8 = np.zeros((128, 8), BF16)
    for nb in range(4):
        i8[32 * nb:32 * nb + 8] = np.eye(8, dtype=BF16)
    wih_pc, bias_pc, whh_pc, hT0_pc, c0_pc, emb_pc = [], [], [], [], [], []
    for c in range(NCORES):
        d = 0 if c < 4 else 1
        W_ih, W_hh = (W_ih_f, W_hh_f) if d == 0 else (W_ih_b, W_hh_b)
        bsum = (np.asarray(b_ih_f) + np.asarray(b_hh_f)) if d == 0 else \
               (np.asarray(b_ih_b) + np.asarray(b_hh_b))
        wih_pc.append(_chunk128(np.asarray(W_ih, F32).T).astype(BF16))
        whh_pc.append(_chunk128(np.asarray(W_hh, F32).T).astype(BF16))
        bias_pc.append(np.asarray(bsum, F32).reshape(1, G).astype(BF16))
        h0 = np.asarray(init_hidden, F32)[d]          # [D]
        hT0 = np.broadcast_to(h0[:, None], (D, BL))   # [D, BL]
        hT0_pc.append(_chunk128(hT0).astype(BF16))
        c0_pc.append(np.broadcast_to(h0[None, :], (BL, D)).astype(F32).copy())
        x = emb if d == 0 else embr
        sl = x[(c % 4) * BL:(c % 4 + 1) * BL]         # [BL, T, D]
        # [D, T, BL] -> [D, T*BL] (t-major, b-minor) -> chunked
        embT = sl.transpose(2, 1, 0).reshape(D, NTOK)
        emb_pc.append(_chunk128(embT).astype(BF16))

    # ---- projections on device ----
    in_maps = [dict(embT=emb_pc[c], wih=wih_pc[c], bias=bias_pc[c], ones=ones)
               for c in range(NCORES)]
    res = run_bass_kernel_spmd(nc_p, in_maps, core_ids=list(range(NCORES)))
    # xp [MT,128,G] -> [T, BL, G]
    xp_pc = [r["xp"].reshape(T, BL, 4, 512).transpose(0, 2, 1, 3).copy()
             for r in res.results]

    # ---- recurrence: NCH sequential chunk calls ----
    hs_pc = [np.empty((T, BL, D), BF16) for _ in range(NCORES)]
    c_st, hT_st = c0_pc, hT0_pc
    for k in range(NCH):
        in_maps = [dict(xpc=np.ascontiguousarray(xp_pc[c][k * CH:(k + 1) * CH]),
                        whh=whh_pc[c], i8=i8, c_in=c_st[c], hT_in=hT_st[c])
                   for c in range(NCORES)]
        res = run_bass_kernel_spmd(nc_r, in_maps, core_ids=list(range(NCORES)))
        for c in range(NCORES):
            hs_pc[c][k * CH:(k + 1) * CH] = res.results[c]["hs"]
        c_st = [res.results[c]["c_out"] for c in range(NCORES)]
        hT_st = [res.results[c]["hT_out"] for c in range(NCORES)]

    # ---- host epilogue ----
    hf = np.concatenate([hs_pc[c].astype(F32) for c in range(4)], axis=1)   # [T,32,D]
    hbr = np.concatenate([hs_pc[c].astype(F32) for c in range(4, 8)], axis=1)
    hf = hf.transpose(1, 0, 2)            # [B,T,D]
    hb = _seq_flip(hbr.transpose(1, 0, 2), lengths)
    feats = np.concatenate([hf, hb], axis=-1)          # [B,T,2D]
    emissions = feats @ np.asarray(W_emit, F32).T + np.asarray(b_emit, F32)

    e = emissions.astype(np.float64)
    tr = np.asarray(trans, np.float64)
    st = np.asarray(start_trans, np.float64)
    et = np.asarray(end_trans, np.float64)
    mask = np.arange(T)[None, :] < lengths[:, None]
    alpha = e[:, 0] + st
    expTrT = np.exp(tr).T  # [j, i]: new_i = LSE_j(alpha_j + tr[i,j])
    for t in range(1, T):
        m = alpha.max(axis=1, keepdims=True)
        new = e[:, t] + m + np.log(np.exp(alpha - m) @ expTrT)
        alpha = np.where(mask[:, t][:, None], new, alpha)
    fwd = _logsumexp(alpha + et, axis=-1)
    e_tag = np.take_along_axis(e, tags[..., None], axis=-1)[..., 0]
    step_scores = tr[tags[:, 1:], tags[:, :-1]] + e_tag[:, 1:]
    last_tag = np.take_along_axis(tags, (lengths - 1)[:, None], axis=1)[:, 0]
    gold = (st[tags[:, 0]] + e_tag[:, 0]
            + np.sum(np.where(mask[:, 1:], step_scores, 0.0), axis=-1)
            + et[last_tag])
    return np.float32(np.sum(fwd - gold))



# revision 7
# speedup vs baseline: 4.3701x; 4.3701x over previous
"""BiLSTM-CRF loss on 8 Trainium2 NeuronCores.

Strategy (v3, fused single kernel):
  - Direction-split: cores 0-3 forward LSTM, cores 4-7 backward LSTM (on
    host-pre-flipped input); batch (32) sharded 4 ways -> 8 sequences/core.
  - Transposed cell layout: gate dim on partitions, (chunk, seq) in the free
    dim. Recurrent + input-projection matmuls all accumulate into one psum
    tile per step (input projection is dependency-free and fills tensor-engine
    idle time, so there is no separate projection kernel and no xp roundtrip).
  - All-tanh gates: i/f/o rows of the weights are pre-scaled by 0.5 so
    sigmoid(x) = 0.5*(tanh(x/2)+1); state is kept as H2=2h (bf16) and C2=2c
    (f32), making the elementwise cell update exact with three fused
    scalar_tensor_tensor ops on gpsimd:
       A = (t_i+1)*t_g ; B = (t_f+1)*C2 ; C2' = 0.5*B + A ; tc = tanh(0.5*C2')
       H2' = (t_o+1)*tc
  - Host (numpy, fp64): embedding gather, sequence flips, emissions, CRF
    forward/gold score.
"""
import sys
import numpy as np

sys.path.insert(0, '/opt/trn_rl_repo')

import concourse.bacc as bacc
import concourse.mybir as mybir
from concourse.tile import TileContext
from concourse.bass_utils import run_bass_kernel_spmd
import ml_dtypes

BF16 = ml_dtypes.bfloat16
F32 = np.float32

B, T = 32, 512
V, D, L = 50257, 512, 48
NCORES = 8
BL = 8          # sequences per core
NM, NK = 16, 4  # gate chunks (128 each), h chunks (128 each)
W = BL * NM     # 128
HC = BL * NK    # 32
CH = T          # steps per kernel call (single call)
HS_BLOCK = 16

_TANH = mybir.ActivationFunctionType.Tanh
_SIG = mybir.ActivationFunctionType.Sigmoid
_ADD = mybir.AluOpType.add
_MULT = mybir.AluOpType.mult

_cache = {}


def _build(ch):
    nc = bacc.Bacc()
    dt = mybir.dt
    embT = nc.declare_dram_parameter("embT", [128, NK * ch * BL], dt.bfloat16,
                                     isOutput=False)
    whh = nc.declare_dram_parameter("whh", [128, NK * NM * 128], dt.bfloat16,
                                    isOutput=False)
    wih = nc.declare_dram_parameter("wih", [128, NK * NM * 128], dt.bfloat16,
                                    isOutput=False)
    biasb = nc.declare_dram_parameter("biasb", [128, W], dt.bfloat16,
                                      isOutput=False)
    ident = nc.declare_dram_parameter("ident", [128, 128], dt.bfloat16,
                                      isOutput=False)
    c_in = nc.declare_dram_parameter("c_in", [128, HC], dt.float32, isOutput=False)
    h_in = nc.declare_dram_parameter("h_in", [128, HC], dt.bfloat16, isOutput=False)
    hs = nc.declare_dram_parameter("hs", [ch // HS_BLOCK, 128, HS_BLOCK * HC],
                                   dt.bfloat16, isOutput=True)

    with TileContext(nc) as tc:
        with (
            tc.tile_pool(name="const", bufs=1) as cpool,
            tc.tile_pool(name="state", bufs=2) as spool,
            tc.tile_pool(name="t", bufs=2) as tpool,
            tc.tile_pool(name="ab", bufs=2) as abpool,
            tc.tile_pool(name="hsb", bufs=2) as hspool,
            tc.tile_pool(name="pg", bufs=4, space="PSUM") as pgpool,
        ):
            embT_sb = cpool.tile([128, NK * ch * BL], dt.bfloat16)
            half = NK * ch * BL // 2
            nc.gpsimd.dma_start(out=embT_sb[:, 0:half], in_=embT[:, 0:half])
            nc.gpsimd.dma_start(out=embT_sb[:, half:], in_=embT[:, half:])
            wih_sb = cpool.tile([128, NK * NM * 128], dt.bfloat16)
            nc.sync.dma_start(out=wih_sb[:, 0:NK * NM * 64],
                              in_=wih[:, 0:NK * NM * 64])
            nc.scalar.dma_start(out=wih_sb[:, NK * NM * 64:],
                                in_=wih[:, NK * NM * 64:])
            whh_sb = cpool.tile([128, NK * NM * 128], dt.bfloat16)
            nc.sync.dma_start(out=whh_sb[:, 0:NK * NM * 64],
                              in_=whh[:, 0:NK * NM * 64])
            nc.scalar.dma_start(out=whh_sb[:, NK * NM * 64:],
                                in_=whh[:, NK * NM * 64:])
            bias_sb = cpool.tile([128, W], dt.bfloat16)
            nc.sync.dma_start(out=bias_sb[:], in_=biasb[:])
            id_sb = cpool.tile([128, 128], dt.bfloat16)
            nc.sync.dma_start(out=id_sb[:], in_=ident[:])
            c_prev = spool.tile([128, HC], dt.float32, tag="c")
            nc.sync.dma_start(out=c_prev[:], in_=c_in[:])
            h_prev = spool.tile([128, HC], dt.bfloat16, tag="h")
            nc.sync.dma_start(out=h_prev[:], in_=h_in[:])

            hs_buf = None
            for j in range(ch):
                pg = pgpool.tile([128, 96], dt.float32, tag="pg")
                pgo = pgpool.tile([128, 32], dt.float32, tag="pgo")
                nc.tensor.matmul(pg[:], id_sb[:], bias_sb[:, 0:96],
                                 start=True, stop=False, skip_group_check=True)
                nc.tensor.matmul(pgo[:], id_sb[:], bias_sb[:, 96:128],
                                 start=True, stop=False, skip_group_check=True)
                for m in range(NM):
                    o = pg[:, m * BL:(m + 1) * BL] if m < 12 else \
                        pgo[:, (m - 12) * BL:(m - 11) * BL]
                    for kc in range(NK):
                        nc.tensor.matmul(
                            o, wih_sb[:, (kc * NM + m) * 128:(kc * NM + m + 1) * 128],
                            embT_sb[:, (kc * ch + j) * BL:(kc * ch + j) * BL + BL],
                            start=False, stop=False, skip_group_check=True)
                for m in range(NM):
                    o = pg[:, m * BL:(m + 1) * BL] if m < 12 else \
                        pgo[:, (m - 12) * BL:(m - 11) * BL]
                    for kc in range(NK):
                        nc.tensor.matmul(
                            o, whh_sb[:, (kc * NM + m) * 128:(kc * NM + m + 1) * 128],
                            h_prev[:, kc * BL:(kc + 1) * BL],
                            start=False, stop=(kc == NK - 1), skip_group_check=True)
                t_sb = tpool.tile([128, W], dt.float32, tag="t")
                nc.scalar.activation(t_sb[:, 0:96], pg[:], _TANH)
                # o gate: direct sigmoid (same act table as tanh, no reload)
                nc.scalar.activation(t_sb[:, 96:128], pgo[:], _SIG)
                # B = (t_f+1)*C2 on DVE first; A = (t_i+1)*t_g as 2 plain
                # tensor-tensor ops on gpsimd (TensorScalarPtr doesn't codegen
                # on Pool)
                b_sb = abpool.tile([128, HC], dt.float32, tag="b", name=f"B_{j}")
                nc.vector.scalar_tensor_tensor(b_sb[:], t_sb[:, 32:64], 1.0,
                                               c_prev[:], _ADD, _MULT)
                ar_sb = abpool.tile([128, HC], dt.float32, tag="ar", name=f"Ar_{j}")
                nc.gpsimd.tensor_mul(ar_sb[:], t_sb[:, 0:32], t_sb[:, 64:96])
                a_sb = abpool.tile([128, HC], dt.float32, tag="a", name=f"A_{j}")
                nc.gpsimd.tensor_add(a_sb[:], ar_sb[:], t_sb[:, 64:96])
                c_new = spool.tile([128, HC], dt.float32, tag="c", name=f"C_{j}")
                nc.vector.scalar_tensor_tensor(c_new[:], b_sb[:], 0.5, a_sb[:],
                                               _MULT, _ADD)
                tc_sb = tpool.tile([128, HC], dt.float32, tag="tc", name=f"TC_{j}")
                nc.scalar.activation(tc_sb[:], c_new[:], _TANH, scale=0.5)
                if j % HS_BLOCK == 0:
                    hs_buf = hspool.tile([128, HS_BLOCK * HC], dt.bfloat16,
                                         tag="hsb")
                h_new = hs_buf[:, (j % HS_BLOCK) * HC:(j % HS_BLOCK + 1) * HC]
                nc.gpsimd.tensor_mul(h_new, t_sb[:, 96:128], tc_sb[:])
                if j % HS_BLOCK == HS_BLOCK - 1:
                    nc.sync.dma_start(out=hs[j // HS_BLOCK], in_=hs_buf[:])
                c_prev, h_prev = c_new, h_new
    nc.finalize()
    return nc


def _pack_w(w):
    """[2048, 512] -> lhsT blocks [128, 64*128]; col (kc*16+m)*128+q =
    w[m*128+q, kc*128+p] at partition p."""
    w4 = np.asarray(w, F32).reshape(NM, 128, NK, 128)   # [m, q, kc, p]
    return np.ascontiguousarray(
        w4.transpose(3, 2, 0, 1).reshape(128, NK * NM * 128)).astype(BF16)


def _pack_x(x):
    """[BL, T, D] -> embT [128, NK*T*BL]; col (kc*T*BL + t*BL + s)."""
    a = np.asarray(x, F32).transpose(2, 1, 0)            # [D, T, BL]
    a = a.reshape(NK, 128, T * BL).transpose(1, 0, 2)    # [128, NK, T*BL]
    return np.ascontiguousarray(a.reshape(128, NK * T * BL)).astype(BF16)


def _seq_flip(x, lengths):
    t = np.arange(x.shape[1])[None, :]
    idx = lengths[:, None] - 1 - t
    idx = np.where(idx >= 0, idx, t)
    return np.take_along_axis(x, idx[:, :, None], axis=1)


def _logsumexp(a, axis):
    m = np.max(a, axis=axis, keepdims=True)
    return np.squeeze(m, axis) + np.log(np.sum(np.exp(a - m), axis=axis))


def kernel(tokens, tags, lengths, embed, W_ih_f, W_hh_f, b_ih_f, b_hh_f,
           W_ih_b, W_hh_b, b_ih_b, b_hh_b, init_hidden, W_emit, b_emit,
           start_trans, trans, end_trans):
    tokens = np.asarray(tokens).astype(np.int64)
    tags = np.asarray(tags).astype(np.int64)
    lengths = np.asarray(lengths).astype(np.int64)
    embed = np.asarray(embed, F32)

    if "rec" not in _cache:
        _cache["rec"] = _build(CH)
    nc = _cache["rec"]

    emb = embed[tokens]                      # [B,T,D] f32
    embr = _seq_flip(emb, lengths)           # reversed input for bwd lstm

    # row scale: i,f gates x0.5 (tanh->sigmoid identity); g, o rows x1
    # (o uses a direct sigmoid activation)
    rs = np.ones((4 * D, 1), F32)
    rs[0:2 * D] = 0.5
    ident = np.eye(128, dtype=BF16)

    in_maps = []
    for c in range(NCORES):
        d = 0 if c < 4 else 1
        W_ih, W_hh = (W_ih_f, W_hh_f) if d == 0 else (W_ih_b, W_hh_b)
        b_sum = (np.asarray(b_ih_f, F32) + np.asarray(b_hh_f, F32)) if d == 0 \
            else (np.asarray(b_ih_b, F32) + np.asarray(b_hh_b, F32))
        wih_p = _pack_w(np.asarray(W_ih, F32) * rs)
        whh_p = _pack_w(np.asarray(W_hh, F32) * rs)
        be = (b_sum[:, None] * rs).reshape(NM, 128).T        # [p, m]
        biasb = np.ascontiguousarray(
            np.repeat(be[:, :, None], BL, axis=2).reshape(128, W)).astype(BF16)
        h0 = np.asarray(init_hidden, F32)[d]                 # [D]
        h0t = np.broadcast_to(h0.reshape(NK, 128).T[:, :, None],
                              (128, NK, BL)).reshape(128, HC)
        c0t = 2.0 * h0t                                      # C2 state = 2c
        x = emb if d == 0 else embr
        sl = x[(c % 4) * BL:(c % 4 + 1) * BL]                # [BL, T, D]
        in_maps.append(dict(
            embT=_pack_x(sl), whh=whh_p, wih=wih_p, biasb=biasb, ident=ident,
            c_in=np.ascontiguousarray(c0t).astype(F32),
            h_in=np.ascontiguousarray(h0t).astype(BF16)))

    res = run_bass_kernel_spmd(nc, in_maps, core_ids=list(range(NCORES)))

    # decode hs: [T/HS, 128, HS, NK, BL] -> h[t, s, kc*128+p]
    h_dec = []
    for c in range(NCORES):
        a = res.results[c]["hs"].reshape(T // HS_BLOCK, 128, HS_BLOCK, NK, BL)
        a = a.transpose(0, 2, 4, 3, 1).reshape(T, BL, D).astype(F32)
        h_dec.append(a)                                      # [T, BL, D]

    hf = np.concatenate([h_dec[c] for c in range(4)], axis=1)      # [T,32,D]
    hbr = np.concatenate([h_dec[c] for c in range(4, 8)], axis=1)
    hf = hf.transpose(1, 0, 2)                                     # [B,T,D]
    hb = _seq_flip(hbr.transpose(1, 0, 2), lengths)
    feats = np.concatenate([hf, hb], axis=-1)                      # [B,T,2D]
    emissions = feats @ np.asarray(W_emit, F32).T + np.asarray(b_emit, F32)

    e = emissions.astype(np.float64)
    tr = np.asarray(trans, np.float64)
    st = np.asarray(start_trans, np.float64)
    et = np.asarray(end_trans, np.float64)
    mask = np.arange(T)[None, :] < lengths[:, None]
    alpha = e[:, 0] + st
    expTrT = np.exp(tr).T
    for t in range(1, T):
        m = alpha.max(axis=1, keepdims=True)
        new = e[:, t] + m + np.log(np.exp(alpha - m) @ expTrT)
        alpha = np.where(mask[:, t][:, None], new, alpha)
    fwd = _logsumexp(alpha + et, axis=-1)
    e_tag = np.take_along_axis(e, tags[..., None], axis=-1)[..., 0]
    step_scores = tr[tags[:, 1:], tags[:, :-1]] + e_tag[:, 1:]
    last_tag = np.take_along_axis(tags, (lengths - 1)[:, None], axis=1)[:, 0]
    gold = (st[tags[:, 0]] + e_tag[:, 0]
            + np.sum(np.where(mask[:, 1:], step_scores, 0.0), axis=-1)
            + et[last_tag])
    return np.float32(np.sum(fwd - gold))


# revision 12
# speedup vs baseline: 4.6053x; 1.0538x over previous
"""BiLSTM-CRF loss on 8 Trainium2 NeuronCores.

Strategy (v3, fused single kernel):
  - Direction-split: cores 0-3 forward LSTM, cores 4-7 backward LSTM (on
    host-pre-flipped input); batch (32) sharded 4 ways -> 8 sequences/core.
  - Transposed cell layout: gate dim on partitions, (chunk, seq) in the free
    dim. Recurrent + input-projection matmuls all accumulate into one psum
    tile per step (input projection is dependency-free and fills tensor-engine
    idle time, so there is no separate projection kernel and no xp roundtrip).
  - All-tanh gates: i/f/o rows of the weights are pre-scaled by 0.5 so
    sigmoid(x) = 0.5*(tanh(x/2)+1); state is kept as H2=2h (bf16) and C2=2c
    (f32), making the elementwise cell update exact with three fused
    scalar_tensor_tensor ops on gpsimd:
       A = (t_i+1)*t_g ; B = (t_f+1)*C2 ; C2' = 0.5*B + A ; tc = tanh(0.5*C2')
       H2' = (t_o+1)*tc
  - Host (numpy, fp64): embedding gather, sequence flips, emissions, CRF
    forward/gold score.
"""
import sys
import numpy as np

sys.path.insert(0, '/opt/trn_rl_repo')

import concourse.bacc as bacc
import concourse.mybir as mybir
from concourse.tile import TileContext
from concourse.bass_utils import run_bass_kernel_spmd
import ml_dtypes

BF16 = ml_dtypes.bfloat16
F32 = np.float32

B, T = 32, 512
V, D, L = 50257, 512, 48
NCORES = 8
BL = 8          # sequences per core
NM, NK = 16, 4  # gate chunks (128 each), h chunks (128 each)
W = BL * NM     # 128
HC = BL * NK    # 32
CH = T          # steps per kernel call (single call)
HS_BLOCK = 16

_TANH = mybir.ActivationFunctionType.Tanh
_SIG = mybir.ActivationFunctionType.Sigmoid
_ADD = mybir.AluOpType.add
_MULT = mybir.AluOpType.mult

_cache = {}


def _build(ch):
    nc = bacc.Bacc()
    dt = mybir.dt
    embT = nc.declare_dram_parameter("embT", [128, NK * ch * BL], dt.bfloat16,
                                     isOutput=False)
    whh = nc.declare_dram_parameter("whh", [128, NK * NM * 128], dt.bfloat16,
                                    isOutput=False)
    wih = nc.declare_dram_parameter("wih", [128, NK * NM * 128], dt.bfloat16,
                                    isOutput=False)
    biasb = nc.declare_dram_parameter("biasb", [128, W], dt.bfloat16,
                                      isOutput=False)
    ident = nc.declare_dram_parameter("ident", [128, 128], dt.bfloat16,
                                      isOutput=False)
    c_in = nc.declare_dram_parameter("c_in", [128, HC], dt.float32, isOutput=False)
    h_in = nc.declare_dram_parameter("h_in", [128, HC], dt.bfloat16, isOutput=False)
    hs = nc.declare_dram_parameter("hs", [ch // HS_BLOCK, 128, HS_BLOCK * HC],
                                   dt.bfloat16, isOutput=True)

    with TileContext(nc) as tc:
        with (
            tc.tile_pool(name="const", bufs=1) as cpool,
            tc.tile_pool(name="state", bufs=2) as spool,
            tc.tile_pool(name="t", bufs=2) as tpool,
            tc.tile_pool(name="ab", bufs=2) as abpool,
            tc.tile_pool(name="hsb", bufs=2) as hspool,
            tc.tile_pool(name="pg", bufs=2, space="PSUM") as pgpool,
        ):
            # small step-0 deps first (DMA latency dominates their cost)
            bias_sb = cpool.tile([128, W], dt.bfloat16)
            nc.sync.dma_start(out=bias_sb[:], in_=biasb[:])
            id_sb = cpool.tile([128, 128], dt.bfloat16)
            nc.sync.dma_start(out=id_sb[:], in_=ident[:])
            c_prev = spool.tile([128, HC], dt.float32, tag="c")
            nc.gpsimd.dma_start(out=c_prev[:], in_=c_in[:])
            h_prev = spool.tile([128, HC], dt.bfloat16, tag="h")
            nc.gpsimd.dma_start(out=h_prev[:], in_=h_in[:])
            # weights split across the two HWDGE queues
            wih_sb = cpool.tile([128, NK * NM * 128], dt.bfloat16)
            nc.sync.dma_start(out=wih_sb[:, 0:NK * NM * 64],
                              in_=wih[:, 0:NK * NM * 64])
            nc.scalar.dma_start(out=wih_sb[:, NK * NM * 64:],
                                in_=wih[:, NK * NM * 64:])
            whh_sb = cpool.tile([128, NK * NM * 128], dt.bfloat16)
            nc.sync.dma_start(out=whh_sb[:, 0:NK * NM * 64],
                              in_=whh[:, 0:NK * NM * 64])
            nc.scalar.dma_start(out=whh_sb[:, NK * NM * 64:],
                                in_=whh[:, NK * NM * 64:])
            # embT: per-k-chunk head (first 64 steps) then tails, so step 0
            # isn't gated on the full 4 MB load
            embT_sb = cpool.tile([128, NK * ch * BL], dt.bfloat16)
            hd = min(64, ch) * BL
            for kc in range(NK):
                nc.gpsimd.dma_start(
                    out=embT_sb[:, kc * ch * BL:kc * ch * BL + hd],
                    in_=embT[:, kc * ch * BL:kc * ch * BL + hd])
            for kc in range(NK):
                if ch * BL > hd:
                    nc.gpsimd.dma_start(
                        out=embT_sb[:, kc * ch * BL + hd:(kc + 1) * ch * BL],
                        in_=embT[:, kc * ch * BL + hd:(kc + 1) * ch * BL])

            # gate chunk m (PyTorch order i0-3 f4-7 g8-11 o12-15) ->
            # (psum tile, col) — g gets its own tile and runs first so its
            # tanh can start while i/f/o matmuls still accumulate.
            def slot(pg_if, pg_g, pg_o, m):
                if m < 8:
                    return pg_if[:, m * BL:(m + 1) * BL]
                if m < 12:
                    return pg_g[:, (m - 8) * BL:(m - 7) * BL]
                return pg_o[:, (m - 12) * BL:(m - 11) * BL]

            MM_ORDER = [8, 9, 10, 11, 0, 1, 2, 3, 4, 5, 6, 7, 12, 13, 14, 15]
            hs_buf = None
            for j in range(ch):
                pg_if = pgpool.tile([128, 64], dt.float32, tag="pgif")
                pg_g = pgpool.tile([128, 32], dt.float32, tag="pgg")
                pg_o = pgpool.tile([128, 32], dt.float32, tag="pgo")
                nc.tensor.matmul(pg_g[:], id_sb[:], bias_sb[:, 64:96],
                                 start=True, stop=False, skip_group_check=True)
                nc.tensor.matmul(pg_if[:], id_sb[:], bias_sb[:, 0:64],
                                 start=True, stop=False, skip_group_check=True)
                nc.tensor.matmul(pg_o[:], id_sb[:], bias_sb[:, 96:128],
                                 start=True, stop=False, skip_group_check=True)
                for m in range(NM):
                    o = slot(pg_if, pg_g, pg_o, m)
                    for kc in range(NK):
                        nc.tensor.matmul(
                            o, wih_sb[:, (kc * NM + m) * 128:(kc * NM + m + 1) * 128],
                            embT_sb[:, (kc * ch + j) * BL:(kc * ch + j) * BL + BL],
                            start=False, stop=False, skip_group_check=True)
                for m in MM_ORDER:
                    o = slot(pg_if, pg_g, pg_o, m)
                    for kc in range(NK):
                        nc.tensor.matmul(
                            o, whh_sb[:, (kc * NM + m) * 128:(kc * NM + m + 1) * 128],
                            h_prev[:, kc * BL:(kc + 1) * BL],
                            start=False, stop=(kc == NK - 1), skip_group_check=True)
                tg_sb = tpool.tile([128, HC], dt.float32, tag="tg")
                nc.scalar.activation(tg_sb[:], pg_g[:], _TANH)
                sif_sb = tpool.tile([128, 64], dt.float32, tag="sif")
                nc.scalar.activation(sif_sb[:], pg_if[:], _SIG)
                so_sb = tpool.tile([128, HC], dt.float32, tag="so")
                nc.scalar.activation(so_sb[:], pg_o[:], _SIG)
                # plain gpsimd tensor ops: b = s_f*c ; a = s_i*t_g ; c' = a+b
                b_sb = abpool.tile([128, HC], dt.float32, tag="b", name=f"B_{j}")
                nc.gpsimd.tensor_mul(b_sb[:], sif_sb[:, 32:64], c_prev[:])
                a_sb = abpool.tile([128, HC], dt.float32, tag="a", name=f"A_{j}")
                nc.gpsimd.tensor_mul(a_sb[:], sif_sb[:, 0:32], tg_sb[:])
                c_new = spool.tile([128, HC], dt.float32, tag="c", name=f"C_{j}")
                nc.gpsimd.tensor_add(c_new[:], a_sb[:], b_sb[:])
                tc_sb = tpool.tile([128, HC], dt.float32, tag="tc", name=f"TC_{j}")
                nc.scalar.activation(tc_sb[:], c_new[:], _TANH)
                if j % HS_BLOCK == 0:
                    hs_buf = hspool.tile([128, HS_BLOCK * HC], dt.bfloat16,
                                         tag="hsb")
                h_new = hs_buf[:, (j % HS_BLOCK) * HC:(j % HS_BLOCK + 1) * HC]
                nc.gpsimd.tensor_mul(h_new, so_sb[:], tc_sb[:])
                if j % HS_BLOCK == HS_BLOCK - 1:
                    nc.sync.dma_start(out=hs[j // HS_BLOCK], in_=hs_buf[:])
                c_prev, h_prev = c_new, h_new
    nc.finalize()
    return nc


def _pack_w(w):
    """[2048, 512] -> lhsT blocks [128, 64*128]; col (kc*16+m)*128+q =
    w[m*128+q, kc*128+p] at partition p."""
    w4 = np.asarray(w, F32).reshape(NM, 128, NK, 128)   # [m, q, kc, p]
    return np.ascontiguousarray(
        w4.transpose(3, 2, 0, 1).reshape(128, NK * NM * 128)).astype(BF16)


def _pack_x(x):
    """[BL, T, D] -> embT [128, NK*T*BL]; col (kc*T*BL + t*BL + s)."""
    a = np.asarray(x, F32).transpose(2, 1, 0)            # [D, T, BL]
    a = a.reshape(NK, 128, T * BL).transpose(1, 0, 2)    # [128, NK, T*BL]
    return np.ascontiguousarray(a.reshape(128, NK * T * BL)).astype(BF16)


def _seq_flip(x, lengths):
    t = np.arange(x.shape[1])[None, :]
    idx = lengths[:, None] - 1 - t
    idx = np.where(idx >= 0, idx, t)
    return np.take_along_axis(x, idx[:, :, None], axis=1)


def _logsumexp(a, axis):
    m = np.max(a, axis=axis, keepdims=True)
    return np.squeeze(m, axis) + np.log(np.sum(np.exp(a - m), axis=axis))


def kernel(tokens, tags, lengths, embed, W_ih_f, W_hh_f, b_ih_f, b_hh_f,
           W_ih_b, W_hh_b, b_ih_b, b_hh_b, init_hidden, W_emit, b_emit,
           start_trans, trans, end_trans):
    tokens = np.asarray(tokens).astype(np.int64)
    tags = np.asarray(tags).astype(np.int64)
    lengths = np.asarray(lengths).astype(np.int64)
    embed = np.asarray(embed, F32)

    if "rec" not in _cache:
        _cache["rec"] = _build(CH)
    nc = _cache["rec"]

    emb = embed[tokens]                      # [B,T,D] f32
    embr = _seq_flip(emb, lengths)           # reversed input for bwd lstm

    ident = np.eye(128, dtype=BF16)

    in_maps = []
    for c in range(NCORES):
        d = 0 if c < 4 else 1
        W_ih, W_hh = (W_ih_f, W_hh_f) if d == 0 else (W_ih_b, W_hh_b)
        b_sum = (np.asarray(b_ih_f, F32) + np.asarray(b_hh_f, F32)) if d == 0 \
            else (np.asarray(b_ih_b, F32) + np.asarray(b_hh_b, F32))
        wih_p = _pack_w(np.asarray(W_ih, F32))
        whh_p = _pack_w(np.asarray(W_hh, F32))
        be = b_sum.reshape(NM, 128).T                        # [p, m]
        biasb = np.ascontiguousarray(
            np.repeat(be[:, :, None], BL, axis=2).reshape(128, W)).astype(BF16)
        h0 = np.asarray(init_hidden, F32)[d]                 # [D]
        h0t = np.broadcast_to(h0.reshape(NK, 128).T[:, :, None],
                              (128, NK, BL)).reshape(128, HC)
        x = emb if d == 0 else embr
        sl = x[(c % 4) * BL:(c % 4 + 1) * BL]                # [BL, T, D]
        in_maps.append(dict(
            embT=_pack_x(sl), whh=whh_p, wih=wih_p, biasb=biasb, ident=ident,
            c_in=np.ascontiguousarray(h0t).astype(F32),
            h_in=np.ascontiguousarray(h0t).astype(BF16)))

    res = run_bass_kernel_spmd(nc, in_maps, core_ids=list(range(NCORES)))

    # decode hs: [T/HS, 128, HS, NK, BL] -> h[t, s, kc*128+p]
    h_dec = []
    for c in range(NCORES):
        a = res.results[c]["hs"].reshape(T // HS_BLOCK, 128, HS_BLOCK, NK, BL)
        a = a.transpose(0, 2, 4, 3, 1).reshape(T, BL, D).astype(F32)
        h_dec.append(a)                                      # [T, BL, D]

    hf = np.concatenate([h_dec[c] for c in range(4)], axis=1)      # [T,32,D]
    hbr = np.concatenate([h_dec[c] for c in range(4, 8)], axis=1)
    hf = hf.transpose(1, 0, 2)                                     # [B,T,D]
    hb = _seq_flip(hbr.transpose(1, 0, 2), lengths)
    feats = np.concatenate([hf, hb], axis=-1)                      # [B,T,2D]
    emissions = feats @ np.asarray(W_emit, F32).T + np.asarray(b_emit, F32)

    e = emissions.astype(np.float64)
    tr = np.asarray(trans, np.float64)
    st = np.asarray(start_trans, np.float64)
    et = np.asarray(end_trans, np.float64)
    mask = np.arange(T)[None, :] < lengths[:, None]
    alpha = e[:, 0] + st
    expTrT = np.exp(tr).T
    for t in range(1, T):
        m = alpha.max(axis=1, keepdims=True)
        new = e[:, t] + m + np.log(np.exp(alpha - m) @ expTrT)
        alpha = np.where(mask[:, t][:, None], new, alpha)
    fwd = _logsumexp(alpha + et, axis=-1)
    e_tag = np.take_along_axis(e, tags[..., None], axis=-1)[..., 0]
    step_scores = tr[tags[:, 1:], tags[:, :-1]] + e_tag[:, 1:]
    last_tag = np.take_along_axis(tags, (lengths - 1)[:, None], axis=1)[:, 0]
    gold = (st[tags[:, 0]] + e_tag[:, 0]
            + np.sum(np.where(mask[:, 1:], step_scores, 0.0), axis=-1)
            + et[last_tag])
    return np.float32(np.sum(fwd - gold))


# revision 14
# speedup vs baseline: 4.6577x; 1.0114x over previous
"""BiLSTM-CRF loss on 8 Trainium2 NeuronCores.

Strategy (v3, fused single kernel):
  - Direction-split: cores 0-3 forward LSTM, cores 4-7 backward LSTM (on
    host-pre-flipped input); batch (32) sharded 4 ways -> 8 sequences/core.
  - Transposed cell layout: gate dim on partitions, (chunk, seq) in the free
    dim. Recurrent + input-projection matmuls all accumulate into one psum
    tile per step (input projection is dependency-free and fills tensor-engine
    idle time, so there is no separate projection kernel and no xp roundtrip).
  - All-tanh gates: i/f/o rows of the weights are pre-scaled by 0.5 so
    sigmoid(x) = 0.5*(tanh(x/2)+1); state is kept as H2=2h (bf16) and C2=2c
    (f32), making the elementwise cell update exact with three fused
    scalar_tensor_tensor ops on gpsimd:
       A = (t_i+1)*t_g ; B = (t_f+1)*C2 ; C2' = 0.5*B + A ; tc = tanh(0.5*C2')
       H2' = (t_o+1)*tc
  - Host (numpy, fp64): embedding gather, sequence flips, emissions, CRF
    forward/gold score.
"""
import sys
import numpy as np

sys.path.insert(0, '/opt/trn_rl_repo')

import concourse.bacc as bacc
import concourse.mybir as mybir
from concourse.tile import TileContext
from concourse.bass_utils import run_bass_kernel_spmd
import ml_dtypes

BF16 = ml_dtypes.bfloat16
F32 = np.float32

B, T = 32, 512
V, D, L = 50257, 512, 48
NCORES = 8
BL = 8          # sequences per core
NM, NK = 16, 4  # gate chunks (128 each), h chunks (128 each)
W = BL * NM     # 128
HC = BL * NK    # 32
CH = T          # steps per kernel call (single call)
HS_BLOCK = 16

_TANH = mybir.ActivationFunctionType.Tanh
_SIG = mybir.ActivationFunctionType.Sigmoid
_ADD = mybir.AluOpType.add
_MULT = mybir.AluOpType.mult

_cache = {}


def _build(ch):
    nc = bacc.Bacc()
    dt = mybir.dt
    embT = nc.declare_dram_parameter("embT", [128, NK * ch * BL], dt.bfloat16,
                                     isOutput=False)
    whh = nc.declare_dram_parameter("whh", [128, NK * NM * 128], dt.bfloat16,
                                    isOutput=False)
    wih = nc.declare_dram_parameter("wih", [128, NK * NM * 128], dt.bfloat16,
                                    isOutput=False)
    biasb = nc.declare_dram_parameter("biasb", [128, W], dt.bfloat16,
                                      isOutput=False)
    ident = nc.declare_dram_parameter("ident", [128, 128], dt.bfloat16,
                                      isOutput=False)
    c_in = nc.declare_dram_parameter("c_in", [128, HC], dt.float32, isOutput=False)
    h_in = nc.declare_dram_parameter("h_in", [128, HC], dt.bfloat16, isOutput=False)
    hs = nc.declare_dram_parameter("hs", [ch // HS_BLOCK, 128, HS_BLOCK * HC],
                                   dt.bfloat16, isOutput=True)

    with TileContext(nc) as tc:
        with (
            tc.tile_pool(name="const", bufs=1) as cpool,
            tc.tile_pool(name="state", bufs=2) as spool,
            tc.tile_pool(name="t", bufs=2) as tpool,
            tc.tile_pool(name="ab", bufs=2) as abpool,
            tc.tile_pool(name="hsb", bufs=2) as hspool,
            tc.tile_pool(name="pg", bufs=2, space="PSUM") as pgpool,
        ):
            # weights go first on the two HWDGE queues (they gate step 0);
            # small tiles + embT ride the Pool SWDGE queue
            wih_sb = cpool.tile([128, NK * NM * 128], dt.bfloat16)
            whh_sb = cpool.tile([128, NK * NM * 128], dt.bfloat16)
            nc.sync.dma_start(out=wih_sb[:, 0:NK * NM * 64],
                              in_=wih[:, 0:NK * NM * 64])
            nc.scalar.dma_start(out=wih_sb[:, NK * NM * 64:],
                                in_=wih[:, NK * NM * 64:])
            nc.sync.dma_start(out=whh_sb[:, 0:NK * NM * 64],
                              in_=whh[:, 0:NK * NM * 64])
            nc.scalar.dma_start(out=whh_sb[:, NK * NM * 64:],
                                in_=whh[:, NK * NM * 64:])
            bias_sb = cpool.tile([128, W], dt.bfloat16)
            nc.gpsimd.dma_start(out=bias_sb[:], in_=biasb[:])
            id_sb = cpool.tile([128, 128], dt.bfloat16)
            nc.gpsimd.dma_start(out=id_sb[:], in_=ident[:])
            c_prev = spool.tile([128, HC], dt.float32, tag="c")
            nc.gpsimd.dma_start(out=c_prev[:], in_=c_in[:])
            h_prev = spool.tile([128, HC], dt.bfloat16, tag="h")
            nc.gpsimd.dma_start(out=h_prev[:], in_=h_in[:])
            # embT: per-k-chunk head (first 64 steps) then tails, so step 0
            # isn't gated on the full 4 MB load
            embT_sb = cpool.tile([128, NK * ch * BL], dt.bfloat16)
            hd = min(64, ch) * BL
            for kc in range(NK):
                nc.gpsimd.dma_start(
                    out=embT_sb[:, kc * ch * BL:kc * ch * BL + hd],
                    in_=embT[:, kc * ch * BL:kc * ch * BL + hd])
            # tails ride the SP queue behind the weights: Pool must stay free
            # for the per-step elementwise ops, Act for the activations
            for kc in range(NK):
                if ch * BL > hd:
                    nc.sync.dma_start(
                        out=embT_sb[:, kc * ch * BL + hd:(kc + 1) * ch * BL],
                        in_=embT[:, kc * ch * BL + hd:(kc + 1) * ch * BL])
            # dummy activation pre-loads the sigmoid/tanh table while the
            # weight DMAs are still in flight
            warm_sb = tpool.tile([1, 1], dt.float32, tag="warm")
            nc.scalar.activation(warm_sb[:], bias_sb[0:1, 0:1], _TANH)

            # gate chunk m (PyTorch order i0-3 f4-7 g8-11 o12-15) ->
            # (psum tile, col) — g gets its own tile and runs first so its
            # tanh can start while i/f/o matmuls still accumulate.
            def slot(pg_if, pg_g, pg_o, m):
                if m < 8:
                    return pg_if[:, m * BL:(m + 1) * BL]
                if m < 12:
                    return pg_g[:, (m - 8) * BL:(m - 7) * BL]
                return pg_o[:, (m - 12) * BL:(m - 11) * BL]

            MM_ORDER = [8, 9, 10, 11, 0, 1, 2, 3, 4, 5, 6, 7, 12, 13, 14, 15]
            hs_buf = None
            for j in range(ch):
                pg_if = pgpool.tile([128, 64], dt.float32, tag="pgif")
                pg_g = pgpool.tile([128, 32], dt.float32, tag="pgg")
                pg_o = pgpool.tile([128, 32], dt.float32, tag="pgo")
                nc.tensor.matmul(pg_g[:], id_sb[:], bias_sb[:, 64:96],
                                 start=True, stop=False, skip_group_check=True)
                nc.tensor.matmul(pg_if[:], id_sb[:], bias_sb[:, 0:64],
                                 start=True, stop=False, skip_group_check=True)
                nc.tensor.matmul(pg_o[:], id_sb[:], bias_sb[:, 96:128],
                                 start=True, stop=False, skip_group_check=True)
                for m in range(NM):
                    o = slot(pg_if, pg_g, pg_o, m)
                    for kc in range(NK):
                        nc.tensor.matmul(
                            o, wih_sb[:, (kc * NM + m) * 128:(kc * NM + m + 1) * 128],
                            embT_sb[:, (kc * ch + j) * BL:(kc * ch + j) * BL + BL],
                            start=False, stop=False, skip_group_check=True)
                for m in MM_ORDER:
                    o = slot(pg_if, pg_g, pg_o, m)
                    for kc in range(NK):
                        nc.tensor.matmul(
                            o, whh_sb[:, (kc * NM + m) * 128:(kc * NM + m + 1) * 128],
                            h_prev[:, kc * BL:(kc + 1) * BL],
                            start=False, stop=(kc == NK - 1), skip_group_check=True)
                tg_sb = tpool.tile([128, HC], dt.float32, tag="tg")
                nc.scalar.activation(tg_sb[:], pg_g[:], _TANH)
                sif_sb = tpool.tile([128, 64], dt.float32, tag="sif")
                nc.scalar.activation(sif_sb[:], pg_if[:], _SIG)
                so_sb = tpool.tile([128, HC], dt.float32, tag="so")
                nc.scalar.activation(so_sb[:], pg_o[:], _SIG)
                # plain gpsimd tensor ops: b = s_f*c ; a = s_i*t_g ; c' = a+b
                b_sb = abpool.tile([128, HC], dt.float32, tag="b", name=f"B_{j}")
                nc.gpsimd.tensor_mul(b_sb[:], sif_sb[:, 32:64], c_prev[:])
                a_sb = abpool.tile([128, HC], dt.float32, tag="a", name=f"A_{j}")
                nc.gpsimd.tensor_mul(a_sb[:], sif_sb[:, 0:32], tg_sb[:])
                c_new = spool.tile([128, HC], dt.float32, tag="c", name=f"C_{j}")
                nc.gpsimd.tensor_add(c_new[:], a_sb[:], b_sb[:])
                tc_sb = tpool.tile([128, HC], dt.float32, tag="tc", name=f"TC_{j}")
                nc.scalar.activation(tc_sb[:], c_new[:], _TANH)
                if j % HS_BLOCK == 0:
                    hs_buf = hspool.tile([128, HS_BLOCK * HC], dt.bfloat16,
                                         tag="hsb")
                h_new = hs_buf[:, (j % HS_BLOCK) * HC:(j % HS_BLOCK + 1) * HC]
                nc.gpsimd.tensor_mul(h_new, so_sb[:], tc_sb[:])
                if j % HS_BLOCK == HS_BLOCK - 1:
                    nc.sync.dma_start(out=hs[j // HS_BLOCK], in_=hs_buf[:])
                c_prev, h_prev = c_new, h_new
    nc.finalize()
    return nc


def _pack_w(w):
    """[2048, 512] -> lhsT blocks [128, 64*128]; col (kc*16+m)*128+q =
    w[m*128+q, kc*128+p] at partition p."""
    w4 = np.asarray(w, F32).reshape(NM, 128, NK, 128)   # [m, q, kc, p]
    return np.ascontiguousarray(
        w4.transpose(3, 2, 0, 1).reshape(128, NK * NM * 128)).astype(BF16)


def _pack_x(x):
    """[BL, T, D] -> embT [128, NK*T*BL]; col (kc*T*BL + t*BL + s)."""
    a = np.asarray(x, F32).transpose(2, 1, 0)            # [D, T, BL]
    a = a.reshape(NK, 128, T * BL).transpose(1, 0, 2)    # [128, NK, T*BL]
    return np.ascontiguousarray(a.reshape(128, NK * T * BL)).astype(BF16)


def _seq_flip(x, lengths):
    t = np.arange(x.shape[1])[None, :]
    idx = lengths[:, None] - 1 - t
    idx = np.where(idx >= 0, idx, t)
    return np.take_along_axis(x, idx[:, :, None], axis=1)


def _logsumexp(a, axis):
    m = np.max(a, axis=axis, keepdims=True)
    return np.squeeze(m, axis) + np.log(np.sum(np.exp(a - m), axis=axis))


def kernel(tokens, tags, lengths, embed, W_ih_f, W_hh_f, b_ih_f, b_hh_f,
           W_ih_b, W_hh_b, b_ih_b, b_hh_b, init_hidden, W_emit, b_emit,
           start_trans, trans, end_trans):
    tokens = np.asarray(tokens).astype(np.int64)
    tags = np.asarray(tags).astype(np.int64)
    lengths = np.asarray(lengths).astype(np.int64)
    embed = np.asarray(embed, F32)

    if "rec" not in _cache:
        _cache["rec"] = _build(CH)
    nc = _cache["rec"]

    emb = embed[tokens]                      # [B,T,D] f32
    embr = _seq_flip(emb, lengths)           # reversed input for bwd lstm

    ident = np.eye(128, dtype=BF16)

    in_maps = []
    for c in range(NCORES):
        d = 0 if c < 4 else 1
        W_ih, W_hh = (W_ih_f, W_hh_f) if d == 0 else (W_ih_b, W_hh_b)
        b_sum = (np.asarray(b_ih_f, F32) + np.asarray(b_hh_f, F32)) if d == 0 \
            else (np.asarray(b_ih_b, F32) + np.asarray(b_hh_b, F32))
        wih_p = _pack_w(np.asarray(W_ih, F32))
        whh_p = _pack_w(np.asarray(W_hh, F32))
        be = b_sum.reshape(NM, 128).T                        # [p, m]
        biasb = np.ascontiguousarray(
            np.repeat(be[:, :, None], BL, axis=2).reshape(128, W)).astype(BF16)
        h0 = np.asarray(init_hidden, F32)[d]                 # [D]
        h0t = np.broadcast_to(h0.reshape(NK, 128).T[:, :, None],
                              (128, NK, BL)).reshape(128, HC)
        x = emb if d == 0 else embr
        sl = x[(c % 4) * BL:(c % 4 + 1) * BL]                # [BL, T, D]
        in_maps.append(dict(
            embT=_pack_x(sl), whh=whh_p, wih=wih_p, biasb=biasb, ident=ident,
            c_in=np.ascontiguousarray(h0t).astype(F32),
            h_in=np.ascontiguousarray(h0t).astype(BF16)))

    res = run_bass_kernel_spmd(nc, in_maps, core_ids=list(range(NCORES)))

    # decode hs: [T/HS, 128, HS, NK, BL] -> h[t, s, kc*128+p]
    h_dec = []
    for c in range(NCORES):
        a = res.results[c]["hs"].reshape(T // HS_BLOCK, 128, HS_BLOCK, NK, BL)
        a = a.transpose(0, 2, 4, 3, 1).reshape(T, BL, D).astype(F32)
        h_dec.append(a)                                      # [T, BL, D]

    hf = np.concatenate([h_dec[c] for c in range(4)], axis=1)      # [T,32,D]
    hbr = np.concatenate([h_dec[c] for c in range(4, 8)], axis=1)
    hf = hf.transpose(1, 0, 2)                                     # [B,T,D]
    hb = _seq_flip(hbr.transpose(1, 0, 2), lengths)
    feats = np.concatenate([hf, hb], axis=-1)                      # [B,T,2D]
    emissions = feats @ np.asarray(W_emit, F32).T + np.asarray(b_emit, F32)

    e = emissions.astype(np.float64)
    tr = np.asarray(trans, np.float64)
    st = np.asarray(start_trans, np.float64)
    et = np.asarray(end_trans, np.float64)
    mask = np.arange(T)[None, :] < lengths[:, None]
    alpha = e[:, 0] + st
    expTrT = np.exp(tr).T
    for t in range(1, T):
        m = alpha.max(axis=1, keepdims=True)
        new = e[:, t] + m + np.log(np.exp(alpha - m) @ expTrT)
        alpha = np.where(mask[:, t][:, None], new, alpha)
    fwd = _logsumexp(alpha + et, axis=-1)
    e_tag = np.take_along_axis(e, tags[..., None], axis=-1)[..., 0]
    step_scores = tr[tags[:, 1:], tags[:, :-1]] + e_tag[:, 1:]
    last_tag = np.take_along_axis(tags, (lengths - 1)[:, None], axis=1)[:, 0]
    gold = (st[tags[:, 0]] + e_tag[:, 0]
            + np.sum(np.where(mask[:, 1:], step_scores, 0.0), axis=-1)
            + et[last_tag])
    return np.float32(np.sum(fwd - gold))


# revision 16
# speedup vs baseline: 4.6621x; 1.0010x over previous
"""BiLSTM-CRF loss on 8 Trainium2 NeuronCores.

Strategy (v3, fused single kernel):
  - Direction-split: cores 0-3 forward LSTM, cores 4-7 backward LSTM (on
    host-pre-flipped input); batch (32) sharded 4 ways -> 8 sequences/core.
  - Transposed cell layout: gate dim on partitions, (chunk, seq) in the free
    dim. Recurrent + input-projection matmuls all accumulate into one psum
    tile per step (input projection is dependency-free and fills tensor-engine
    idle time, so there is no separate projection kernel and no xp roundtrip).
  - All-tanh gates: i/f/o rows of the weights are pre-scaled by 0.5 so
    sigmoid(x) = 0.5*(tanh(x/2)+1); state is kept as H2=2h (bf16) and C2=2c
    (f32), making the elementwise cell update exact with three fused
    scalar_tensor_tensor ops on gpsimd:
       A = (t_i+1)*t_g ; B = (t_f+1)*C2 ; C2' = 0.5*B + A ; tc = tanh(0.5*C2')
       H2' = (t_o+1)*tc
  - Host (numpy, fp64): embedding gather, sequence flips, emissions, CRF
    forward/gold score.
"""
import sys
import numpy as np

sys.path.insert(0, '/opt/trn_rl_repo')

import concourse.bacc as bacc
import concourse.mybir as mybir
from concourse.tile import TileContext
from concourse.bass_utils import run_bass_kernel_spmd
import ml_dtypes

BF16 = ml_dtypes.bfloat16
F32 = np.float32

B, T = 32, 512
V, D, L = 50257, 512, 48
NCORES = 8
BL = 8          # sequences per core
NM, NK = 16, 4  # gate chunks (128 each), h chunks (128 each)
W = BL * NM     # 128
HC = BL * NK    # 32
CH = T          # steps per kernel call (single call)
HS_BLOCK = 16

_TANH = mybir.ActivationFunctionType.Tanh
_SIG = mybir.ActivationFunctionType.Sigmoid
_ADD = mybir.AluOpType.add
_MULT = mybir.AluOpType.mult

_cache = {}


def _build(ch):
    nc = bacc.Bacc()
    dt = mybir.dt
    embT = nc.declare_dram_parameter("embT", [128, NK * ch * BL], dt.bfloat16,
                                     isOutput=False)
    whh = nc.declare_dram_parameter("whh", [128, NK * NM * 128], dt.bfloat16,
                                    isOutput=False)
    wih = nc.declare_dram_parameter("wih", [128, NK * NM * 128], dt.bfloat16,
                                    isOutput=False)
    biasb = nc.declare_dram_parameter("biasb", [128, W], dt.bfloat16,
                                      isOutput=False)
    ident = nc.declare_dram_parameter("ident", [128, 128], dt.bfloat16,
                                      isOutput=False)
    c_in = nc.declare_dram_parameter("c_in", [128, HC], dt.float32, isOutput=False)
    h_in = nc.declare_dram_parameter("h_in", [128, HC], dt.bfloat16, isOutput=False)
    hs = nc.declare_dram_parameter("hs", [ch // HS_BLOCK, 128, HS_BLOCK * HC],
                                   dt.bfloat16, isOutput=True)

    with TileContext(nc) as tc:
        with (
            tc.tile_pool(name="const", bufs=1) as cpool,
            tc.tile_pool(name="state", bufs=2) as spool,
            tc.tile_pool(name="t", bufs=2) as tpool,
            tc.tile_pool(name="ab", bufs=2) as abpool,
            tc.tile_pool(name="hsb", bufs=2) as hspool,
            tc.tile_pool(name="pg", bufs=2, space="PSUM") as pgpool,
        ):
            # balanced 3-way load: both HWDGE queues carry wih + 5/13 of whh
            # each; the Pool SWDGE queue carries the small tiles, the embT
            # heads, and the remaining 3/13 of whh, so all three queues drain
            # at roughly the same time (step 0 is gated on the last weight
            # byte)
            wih_sb = cpool.tile([128, NK * NM * 128], dt.bfloat16)
            whh_sb = cpool.tile([128, NK * NM * 128], dt.bfloat16)
            WTOT = NK * NM * 128
            s1 = (WTOT * 5) // 13
            nc.sync.dma_start(out=wih_sb[:, 0:WTOT // 2],
                              in_=wih[:, 0:WTOT // 2])
            nc.scalar.dma_start(out=wih_sb[:, WTOT // 2:],
                                in_=wih[:, WTOT // 2:])
            nc.sync.dma_start(out=whh_sb[:, 0:s1], in_=whh[:, 0:s1])
            nc.scalar.dma_start(out=whh_sb[:, s1:2 * s1], in_=whh[:, s1:2 * s1])
            bias_sb = cpool.tile([128, W], dt.bfloat16)
            nc.gpsimd.dma_start(out=bias_sb[:], in_=biasb[:])
            id_sb = cpool.tile([128, 128], dt.bfloat16)
            nc.gpsimd.dma_start(out=id_sb[:], in_=ident[:])
            c_prev = spool.tile([128, HC], dt.float32, tag="c")
            nc.gpsimd.dma_start(out=c_prev[:], in_=c_in[:])
            h_prev = spool.tile([128, HC], dt.bfloat16, tag="h")
            nc.gpsimd.dma_start(out=h_prev[:], in_=h_in[:])
            # embT: per-k-chunk head (first 64 steps) then tails, so step 0
            # isn't gated on the full 4 MB load
            embT_sb = cpool.tile([128, NK * ch * BL], dt.bfloat16)
            hd = min(64, ch) * BL
            for kc in range(NK):
                nc.gpsimd.dma_start(
                    out=embT_sb[:, kc * ch * BL:kc * ch * BL + hd],
                    in_=embT[:, kc * ch * BL:kc * ch * BL + hd])
            # Pool's share of whh rides after the heads
            nc.gpsimd.dma_start(out=whh_sb[:, 2 * s1:], in_=whh[:, 2 * s1:])
            # tails ride the SP queue behind the weights: Pool must stay free
            # for the per-step elementwise ops, Act for the activations
            for kc in range(NK):
                if ch * BL > hd:
                    nc.sync.dma_start(
                        out=embT_sb[:, kc * ch * BL + hd:(kc + 1) * ch * BL],
                        in_=embT[:, kc * ch * BL + hd:(kc + 1) * ch * BL])
            # dummy activation pre-loads the sigmoid/tanh table while the
            # weight DMAs are still in flight
            warm_sb = tpool.tile([1, 1], dt.float32, tag="warm")
            nc.scalar.activation(warm_sb[:], bias_sb[0:1, 0:1], _TANH)

            # gate chunk m (PyTorch order i0-3 f4-7 g8-11 o12-15) ->
            # (psum tile, col) — g gets its own tile and runs first so its
            # tanh can start while i/f/o matmuls still accumulate.
            def slot(pg_if, pg_g, pg_o, m):
                if m < 8:
                    return pg_if[:, m * BL:(m + 1) * BL]
                if m < 12:
                    return pg_g[:, (m - 8) * BL:(m - 7) * BL]
                return pg_o[:, (m - 12) * BL:(m - 11) * BL]

            MM_ORDER = [8, 9, 10, 11, 0, 1, 2, 3, 4, 5, 6, 7, 12, 13, 14, 15]
            hs_buf = None
            for j in range(ch):
                pg_if = pgpool.tile([128, 64], dt.float32, tag="pgif")
                pg_g = pgpool.tile([128, 32], dt.float32, tag="pgg")
                pg_o = pgpool.tile([128, 32], dt.float32, tag="pgo")
                nc.tensor.matmul(pg_g[:], id_sb[:], bias_sb[:, 64:96],
                                 start=True, stop=False, skip_group_check=True)
                nc.tensor.matmul(pg_if[:], id_sb[:], bias_sb[:, 0:64],
                                 start=True, stop=False, skip_group_check=True)
                nc.tensor.matmul(pg_o[:], id_sb[:], bias_sb[:, 96:128],
                                 start=True, stop=False, skip_group_check=True)
                for m in range(NM):
                    o = slot(pg_if, pg_g, pg_o, m)
                    for kc in range(NK):
                        nc.tensor.matmul(
                            o, wih_sb[:, (kc * NM + m) * 128:(kc * NM + m + 1) * 128],
                            embT_sb[:, (kc * ch + j) * BL:(kc * ch + j) * BL + BL],
                            start=False, stop=False, skip_group_check=True)
                for m in MM_ORDER:
                    o = slot(pg_if, pg_g, pg_o, m)
                    for kc in range(NK):
                        nc.tensor.matmul(
                            o, whh_sb[:, (kc * NM + m) * 128:(kc * NM + m + 1) * 128],
                            h_prev[:, kc * BL:(kc + 1) * BL],
                            start=False, stop=(kc == NK - 1), skip_group_check=True)
                tg_sb = tpool.tile([128, HC], dt.float32, tag="tg")
                nc.scalar.activation(tg_sb[:], pg_g[:], _TANH)
                sif_sb = tpool.tile([128, 64], dt.float32, tag="sif")
                nc.scalar.activation(sif_sb[:], pg_if[:], _SIG)
                so_sb = tpool.tile([128, HC], dt.float32, tag="so")
                nc.scalar.activation(so_sb[:], pg_o[:], _SIG)
                # plain gpsimd tensor ops: b = s_f*c ; a = s_i*t_g ; c' = a+b
                b_sb = abpool.tile([128, HC], dt.float32, tag="b", name=f"B_{j}")
                nc.gpsimd.tensor_mul(b_sb[:], sif_sb[:, 32:64], c_prev[:])
                a_sb = abpool.tile([128, HC], dt.float32, tag="a", name=f"A_{j}")
                nc.gpsimd.tensor_mul(a_sb[:], sif_sb[:, 0:32], tg_sb[:])
                c_new = spool.tile([128, HC], dt.float32, tag="c", name=f"C_{j}")
                nc.gpsimd.tensor_add(c_new[:], a_sb[:], b_sb[:])
                tc_sb = tpool.tile([128, HC], dt.float32, tag="tc", name=f"TC_{j}")
                nc.scalar.activation(tc_sb[:], c_new[:], _TANH)
                if j % HS_BLOCK == 0:
                    hs_buf = hspool.tile([128, HS_BLOCK * HC], dt.bfloat16,
                                         tag="hsb")
                h_new = hs_buf[:, (j % HS_BLOCK) * HC:(j % HS_BLOCK + 1) * HC]
                nc.gpsimd.tensor_mul(h_new, so_sb[:], tc_sb[:])
                if j % HS_BLOCK == HS_BLOCK - 1:
                    nc.sync.dma_start(out=hs[j // HS_BLOCK], in_=hs_buf[:])
                c_prev, h_prev = c_new, h_new
    nc.finalize()
    return nc


def _pack_w(w):
    """[2048, 512] -> lhsT blocks [128, 64*128]; col (kc*16+m)*128+q =
    w[m*128+q, kc*128+p] at partition p."""
    w4 = np.asarray(w, F32).reshape(NM, 128, NK, 128)   # [m, q, kc, p]
    return np.ascontiguousarray(
        w4.transpose(3, 2, 0, 1).reshape(128, NK * NM * 128)).astype(BF16)


def _pack_x(x):
    """[BL, T, D] -> embT [128, NK*T*BL]; col (kc*T*BL + t*BL + s)."""
    a = np.asarray(x, F32).transpose(2, 1, 0)            # [D, T, BL]
    a = a.reshape(NK, 128, T * BL).transpose(1, 0, 2)    # [128, NK, T*BL]
    return np.ascontiguousarray(a.reshape(128, NK * T * BL)).astype(BF16)


def _seq_flip(x, lengths):
    t = np.arange(x.shape[1])[None, :]
    idx = lengths[:, None] - 1 - t
    idx = np.where(idx >= 0, idx, t)
    return np.take_along_axis(x, idx[:, :, None], axis=1)


def _logsumexp(a, axis):
    m = np.max(a, axis=axis, keepdims=True)
    return np.squeeze(m, axis) + np.log(np.sum(np.exp(a - m), axis=axis))


def kernel(tokens, tags, lengths, embed, W_ih_f, W_hh_f, b_ih_f, b_hh_f,
           W_ih_b, W_hh_b, b_ih_b, b_hh_b, init_hidden, W_emit, b_emit,
           start_trans, trans, end_trans):
    tokens = np.asarray(tokens).astype(np.int64)
    tags = np.asarray(tags).astype(np.int64)
    lengths = np.asarray(lengths).astype(np.int64)
    embed = np.asarray(embed, F32)

    if "rec" not in _cache:
        _cache["rec"] = _build(CH)
    nc = _cache["rec"]

    emb = embed[tokens]                      # [B,T,D] f32
    embr = _seq_flip(emb, lengths)           # reversed input for bwd lstm

    ident = np.eye(128, dtype=BF16)

    in_maps = []
    for c in range(NCORES):
        d = 0 if c < 4 else 1
        W_ih, W_hh = (W_ih_f, W_hh_f) if d == 0 else (W_ih_b, W_hh_b)
        b_sum = (np.asarray(b_ih_f, F32) + np.asarray(b_hh_f, F32)) if d == 0 \
            else (np.asarray(b_ih_b, F32) + np.asarray(b_hh_b, F32))
        wih_p = _pack_w(np.asarray(W_ih, F32))
        whh_p = _pack_w(np.asarray(W_hh, F32))
        be = b_sum.reshape(NM, 128).T                        # [p, m]
        biasb = np.ascontiguousarray(
            np.repeat(be[:, :, None], BL, axis=2).reshape(128, W)).astype(BF16)
        h0 = np.asarray(init_hidden, F32)[d]                 # [D]
        h0t = np.broadcast_to(h0.reshape(NK, 128).T[:, :, None],
                              (128, NK, BL)).reshape(128, HC)
        x = emb if d == 0 else embr
        sl = x[(c % 4) * BL:(c % 4 + 1) * BL]                # [BL, T, D]
        in_maps.append(dict(
            embT=_pack_x(sl), whh=whh_p, wih=wih_p, biasb=biasb, ident=ident,
            c_in=np.ascontiguousarray(h0t).astype(F32),
            h_in=np.ascontiguousarray(h0t).astype(BF16)))

    res = run_bass_kernel_spmd(nc, in_maps, core_ids=list(range(NCORES)))

    # decode hs: [T/HS, 128, HS, NK, BL] -> h[t, s, kc*128+p]
    h_dec = []
    for c in range(NCORES):
        a = res.results[c]["hs"].reshape(T // HS_BLOCK, 128, HS_BLOCK, NK, BL)
        a = a.transpose(0, 2, 4, 3, 1).reshape(T, BL, D).astype(F32)
        h_dec.append(a)                                      # [T, BL, D]

    hf = np.concatenate([h_dec[c] for c in range(4)], axis=1)      # [T,32,D]
    hbr = np.concatenate([h_dec[c] for c in range(4, 8)], axis=1)
    hf = hf.transpose(1, 0, 2)                                     # [B,T,D]
    hb = _seq_flip(hbr.transpose(1, 0, 2), lengths)
    feats = np.concatenate([hf, hb], axis=-1)                      # [B,T,2D]
    emissions = feats @ np.asarray(W_emit, F32).T + np.asarray(b_emit, F32)

    e = emissions.astype(np.float64)
    tr = np.asarray(trans, np.float64)
    st = np.asarray(start_trans, np.float64)
    et = np.asarray(end_trans, np.float64)
    mask = np.arange(T)[None, :] < lengths[:, None]
    alpha = e[:, 0] + st
    expTrT = np.exp(tr).T
    for t in range(1, T):
        m = alpha.max(axis=1, keepdims=True)
        new = e[:, t] + m + np.log(np.exp(alpha - m) @ expTrT)
        alpha = np.where(mask[:, t][:, None], new, alpha)
    fwd = _logsumexp(alpha + et, axis=-1)
    e_tag = np.take_along_axis(e, tags[..., None], axis=-1)[..., 0]
    step_scores = tr[tags[:, 1:], tags[:, :-1]] + e_tag[:, 1:]
    last_tag = np.take_along_axis(tags, (lengths - 1)[:, None], axis=1)[:, 0]
    gold = (st[tags[:, 0]] + e_tag[:, 0]
            + np.sum(np.where(mask[:, 1:], step_scores, 0.0), axis=-1)
            + et[last_tag])
    return np.float32(np.sum(fwd - gold))


# revision 20
# speedup vs baseline: 4.7616x; 1.0213x over previous
"""BiLSTM-CRF loss on 8 Trainium2 NeuronCores.

Strategy (v3, fused single kernel):
  - Direction-split: cores 0-3 forward LSTM, cores 4-7 backward LSTM (on
    host-pre-flipped input); batch (32) sharded 4 ways -> 8 sequences/core.
  - Transposed cell layout: gate dim on partitions, (chunk, seq) in the free
    dim. Recurrent + input-projection matmuls all accumulate into one psum
    tile per step (input projection is dependency-free and fills tensor-engine
    idle time, so there is no separate projection kernel and no xp roundtrip).
  - All-tanh gates: i/f/o rows of the weights are pre-scaled by 0.5 so
    sigmoid(x) = 0.5*(tanh(x/2)+1); state is kept as H2=2h (bf16) and C2=2c
    (f32), making the elementwise cell update exact with three fused
    scalar_tensor_tensor ops on gpsimd:
       A = (t_i+1)*t_g ; B = (t_f+1)*C2 ; C2' = 0.5*B + A ; tc = tanh(0.5*C2')
       H2' = (t_o+1)*tc
  - Host (numpy, fp64): embedding gather, sequence flips, emissions, CRF
    forward/gold score.
"""
import sys
import numpy as np

sys.path.insert(0, '/opt/trn_rl_repo')

import concourse.bacc as bacc
import concourse.mybir as mybir
from concourse.tile import TileContext
from concourse.bass_utils import run_bass_kernel_spmd
import ml_dtypes

BF16 = ml_dtypes.bfloat16
F32 = np.float32

B, T = 32, 512
V, D, L = 50257, 512, 48
NCORES = 8
BL = 8          # sequences per core
NM, NK = 16, 4  # gate chunks (128 each), h chunks (128 each)
W = BL * NM     # 128
HC = BL * NK    # 32
CH = T          # steps per kernel call (single call)
HS_BLOCK = 16

_TANH = mybir.ActivationFunctionType.Tanh
_SIG = mybir.ActivationFunctionType.Sigmoid
_ADD = mybir.AluOpType.add
_MULT = mybir.AluOpType.mult

_cache = {}


def _build(ch):
    nc = bacc.Bacc()
    dt = mybir.dt
    embT = nc.declare_dram_parameter("embT", [128, NK * ch * BL], dt.bfloat16,
                                     isOutput=False)
    whh = nc.declare_dram_parameter("whh", [128, NK * NM * 128], dt.bfloat16,
                                    isOutput=False)
    wih = nc.declare_dram_parameter("wih", [128, NK * NM * 128], dt.bfloat16,
                                    isOutput=False)
    biasb = nc.declare_dram_parameter("biasb", [128, W], dt.bfloat16,
                                      isOutput=False)
    ident = nc.declare_dram_parameter("ident", [128, 128], dt.bfloat16,
                                      isOutput=False)
    c_in = nc.declare_dram_parameter("c_in", [128, HC], dt.float32, isOutput=False)
    h_in = nc.declare_dram_parameter("h_in", [128, HC], dt.bfloat16, isOutput=False)
    hs = nc.declare_dram_parameter("hs", [ch // HS_BLOCK, 128, HS_BLOCK * HC],
                                   dt.bfloat16, isOutput=True)

    with TileContext(nc) as tc:
        with (
            tc.tile_pool(name="const", bufs=1) as cpool,
            tc.tile_pool(name="state", bufs=2) as spool,
            tc.tile_pool(name="t", bufs=2) as tpool,
            tc.tile_pool(name="ab", bufs=2) as abpool,
            tc.tile_pool(name="hsb", bufs=2) as hspool,
            tc.tile_pool(name="pg", bufs=2, space="PSUM") as pgpool,
        ):
            # balanced 3-way load: both HWDGE queues carry wih + 5/13 of whh
            # each; the Pool SWDGE queue carries the small tiles, the embT
            # heads, and the remaining 3/13 of whh, so all three queues drain
            # at roughly the same time (step 0 is gated on the last weight
            # byte)
            wih_sb = cpool.tile([128, NK * NM * 128], dt.bfloat16)
            whh_sb = cpool.tile([128, NK * NM * 128], dt.bfloat16)
            WTOT = NK * NM * 128
            s1 = (WTOT * 5) // 13
            nc.sync.dma_start(out=wih_sb[:, 0:WTOT // 2],
                              in_=wih[:, 0:WTOT // 2])
            nc.scalar.dma_start(out=wih_sb[:, WTOT // 2:],
                                in_=wih[:, WTOT // 2:])
            nc.sync.dma_start(out=whh_sb[:, 0:s1], in_=whh[:, 0:s1])
            nc.scalar.dma_start(out=whh_sb[:, s1:2 * s1], in_=whh[:, s1:2 * s1])
            bias_sb = cpool.tile([128, W], dt.bfloat16)
            nc.gpsimd.dma_start(out=bias_sb[:], in_=biasb[:])
            id_sb = cpool.tile([128, 128], dt.bfloat16)
            nc.gpsimd.dma_start(out=id_sb[:], in_=ident[:])
            c_prev = spool.tile([128, HC], dt.float32, tag="c")
            nc.gpsimd.dma_start(out=c_prev[:], in_=c_in[:])
            h_prev = spool.tile([128, HC], dt.bfloat16, tag="h")
            nc.gpsimd.dma_start(out=h_prev[:], in_=h_in[:])
            # embT: per-k-chunk head (first 64 steps) then tails, so step 0
            # isn't gated on the full 4 MB load
            embT_sb = cpool.tile([128, NK * ch * BL], dt.bfloat16)
            hd = min(64, ch) * BL
            for kc in range(NK):
                nc.gpsimd.dma_start(
                    out=embT_sb[:, kc * ch * BL:kc * ch * BL + hd],
                    in_=embT[:, kc * ch * BL:kc * ch * BL + hd])
            # Pool's share of whh rides after the heads
            nc.gpsimd.dma_start(out=whh_sb[:, 2 * s1:], in_=whh[:, 2 * s1:])
            # tails ride the SP queue behind the weights: Pool must stay free
            # for the per-step elementwise ops, Act for the activations
            for kc in range(NK):
                if ch * BL > hd:
                    nc.sync.dma_start(
                        out=embT_sb[:, kc * ch * BL + hd:(kc + 1) * ch * BL],
                        in_=embT[:, kc * ch * BL + hd:(kc + 1) * ch * BL])
            # dummy activation pre-loads the sigmoid/tanh table while the
            # weight DMAs are still in flight
            warm_sb = tpool.tile([1, 1], dt.float32, tag="warm")
            nc.scalar.activation(warm_sb[:], bias_sb[0:1, 0:1], _TANH)

            # gate chunk m (PyTorch order i0-3 f4-7 g8-11 o12-15) ->
            # (psum tile, col) — g gets its own tile and runs first so its
            # tanh can start while i/f/o matmuls still accumulate.
            def slot(pg_if, pg_g, pg_o, m):
                if m < 8:
                    return pg_if[:, m * BL:(m + 1) * BL]
                if m < 12:
                    return pg_g[:, (m - 8) * BL:(m - 7) * BL]
                return pg_o[:, (m - 12) * BL:(m - 11) * BL]

            MM_ORDER = [8, 9, 10, 11, 0, 1, 2, 3, 4, 5, 6, 7, 12, 13, 14, 15]
            hs_buf = None
            for j in range(ch):
                pg_if = pgpool.tile([128, 64], dt.float32, tag="pgif")
                pg_g = pgpool.tile([128, 32], dt.float32, tag="pgg")
                pg_o = pgpool.tile([128, 32], dt.float32, tag="pgo")
                nc.tensor.matmul(pg_g[:], id_sb[:], bias_sb[:, 64:96],
                                 start=True, stop=False, skip_group_check=True)
                nc.tensor.matmul(pg_if[:], id_sb[:], bias_sb[:, 0:64],
                                 start=True, stop=False, skip_group_check=True)
                nc.tensor.matmul(pg_o[:], id_sb[:], bias_sb[:, 96:128],
                                 start=True, stop=False, skip_group_check=True)
                for m in range(NM):
                    o = slot(pg_if, pg_g, pg_o, m)
                    for kc in range(NK):
                        nc.tensor.matmul(
                            o, wih_sb[:, (kc * NM + m) * 128:(kc * NM + m + 1) * 128],
                            embT_sb[:, (kc * ch + j) * BL:(kc * ch + j) * BL + BL],
                            start=False, stop=False, skip_group_check=True)
                # g-gate matmuls in kc waves (h arrives in two halves, so the
                # kc0/1 wave starts one Pool-op earlier); i/f/o follow
                for kcs, ms in (([0, 1], [8, 9, 10, 11]),
                                ([2, 3], [8, 9, 10, 11]),
                                (range(NK), [0, 1, 2, 3, 4, 5, 6, 7,
                                             12, 13, 14, 15])):
                    for m in ms:
                        o = slot(pg_if, pg_g, pg_o, m)
                        for kc in kcs:
                            nc.tensor.matmul(
                                o, whh_sb[:, (kc * NM + m) * 128:(kc * NM + m + 1) * 128],
                                h_prev[:, kc * BL:(kc + 1) * BL],
                                start=False, stop=(kc == NK - 1),
                                skip_group_check=True)
                tg_sb = tpool.tile([128, HC], dt.float32, tag="tg")
                nc.scalar.activation(tg_sb[:], pg_g[:], _TANH)
                sif_sb = tpool.tile([128, 64], dt.float32, tag="sif")
                nc.scalar.activation(sif_sb[:], pg_if[:], _SIG)
                so_sb = tpool.tile([128, HC], dt.float32, tag="so")
                nc.scalar.activation(so_sb[:], pg_o[:], _SIG)
                # plain gpsimd tensor ops, split in halves so each half of
                # c' = s_i*t_g + s_f*c can retire one Pool-op earlier
                hh = HC // 2
                b_sb = abpool.tile([128, HC], dt.float32, tag="b", name=f"B_{j}")
                a_sb = abpool.tile([128, HC], dt.float32, tag="a", name=f"A_{j}")
                c_new = spool.tile([128, HC], dt.float32, tag="c", name=f"C_{j}")
                nc.gpsimd.tensor_mul(a_sb[:, 0:hh], sif_sb[:, 0:hh],
                                     tg_sb[:, 0:hh])
                nc.gpsimd.tensor_mul(b_sb[:, 0:hh], sif_sb[:, 32:32 + hh],
                                     c_prev[:, 0:hh])
                nc.gpsimd.tensor_mul(a_sb[:, hh:HC], sif_sb[:, hh:2 * hh],
                                     tg_sb[:, hh:HC])
                nc.gpsimd.tensor_mul(b_sb[:, hh:HC], sif_sb[:, 32 + hh:64],
                                     c_prev[:, hh:HC])
                nc.gpsimd.tensor_add(c_new[:, 0:hh], a_sb[:, 0:hh],
                                     b_sb[:, 0:hh])
                nc.gpsimd.tensor_add(c_new[:, hh:HC], a_sb[:, hh:HC],
                                     b_sb[:, hh:HC])
                tc_sb = tpool.tile([128, HC], dt.float32, tag="tc", name=f"TC_{j}")
                nc.scalar.activation(tc_sb[:], c_new[:], _TANH)
                if j % HS_BLOCK == 0:
                    hs_buf = hspool.tile([128, HS_BLOCK * HC], dt.bfloat16,
                                         tag="hsb")
                base = (j % HS_BLOCK) * HC
                hh = HC // 2
                nc.gpsimd.tensor_mul(hs_buf[:, base:base + hh],
                                     so_sb[:, 0:hh], tc_sb[:, 0:hh])
                nc.gpsimd.tensor_mul(hs_buf[:, base + hh:base + HC],
                                     so_sb[:, hh:HC], tc_sb[:, hh:HC])
                h_new = hs_buf[:, base:base + HC]
                if j % HS_BLOCK == HS_BLOCK - 1:
                    nc.sync.dma_start(out=hs[j // HS_BLOCK], in_=hs_buf[:])
                c_prev, h_prev = c_new, h_new
    nc.finalize()
    return nc


def _pack_w(w):
    """[2048, 512] -> lhsT blocks [128, 64*128]; col (kc*16+m)*128+q =
    w[m*128+q, kc*128+p] at partition p."""
    w4 = np.asarray(w, F32).reshape(NM, 128, NK, 128)   # [m, q, kc, p]
    return np.ascontiguousarray(
        w4.transpose(3, 2, 0, 1).reshape(128, NK * NM * 128)).astype(BF16)


def _pack_x(x):
    """[BL, T, D] -> embT [128, NK*T*BL]; col (kc*T*BL + t*BL + s)."""
    a = np.asarray(x, F32).transpose(2, 1, 0)            # [D, T, BL]
    a = a.reshape(NK, 128, T * BL).transpose(1, 0, 2)    # [128, NK, T*BL]
    return np.ascontiguousarray(a.reshape(128, NK * T * BL)).astype(BF16)


def _seq_flip(x, lengths):
    t = np.arange(x.shape[1])[None, :]
    idx = lengths[:, None] - 1 - t
    idx = np.where(idx >= 0, idx, t)
    return np.take_along_axis(x, idx[:, :, None], axis=1)


def _logsumexp(a, axis):
    m = np.max(a, axis=axis, keepdims=True)
    return np.squeeze(m, axis) + np.log(np.sum(np.exp(a - m), axis=axis))


def kernel(tokens, tags, lengths, embed, W_ih_f, W_hh_f, b_ih_f, b_hh_f,
           W_ih_b, W_hh_b, b_ih_b, b_hh_b, init_hidden, W_emit, b_emit,
           start_trans, trans, end_trans):
    tokens = np.asarray(tokens).astype(np.int64)
    tags = np.asarray(tags).astype(np.int64)
    lengths = np.asarray(lengths).astype(np.int64)
    embed = np.asarray(embed, F32)

    if "rec" not in _cache:
        _cache["rec"] = _build(CH)
    nc = _cache["rec"]

    emb = embed[tokens]                      # [B,T,D] f32
    embr = _seq_flip(emb, lengths)           # reversed input for bwd lstm

    ident = np.eye(128, dtype=BF16)

    in_maps = []
    for c in range(NCORES):
        d = 0 if c < 4 else 1
        W_ih, W_hh = (W_ih_f, W_hh_f) if d == 0 else (W_ih_b, W_hh_b)
        b_sum = (np.asarray(b_ih_f, F32) + np.asarray(b_hh_f, F32)) if d == 0 \
            else (np.asarray(b_ih_b, F32) + np.asarray(b_hh_b, F32))
        wih_p = _pack_w(np.asarray(W_ih, F32))
        whh_p = _pack_w(np.asarray(W_hh, F32))
        be = b_sum.reshape(NM, 128).T                        # [p, m]
        biasb = np.ascontiguousarray(
            np.repeat(be[:, :, None], BL, axis=2).reshape(128, W)).astype(BF16)
        h0 = np.asarray(init_hidden, F32)[d]                 # [D]
        h0t = np.broadcast_to(h0.reshape(NK, 128).T[:, :, None],
                              (128, NK, BL)).reshape(128, HC)
        x = emb if d == 0 else embr
        sl = x[(c % 4) * BL:(c % 4 + 1) * BL]                # [BL, T, D]
        in_maps.append(dict(
            embT=_pack_x(sl), whh=whh_p, wih=wih_p, biasb=biasb, ident=ident,
            c_in=np.ascontiguousarray(h0t).astype(F32),
            h_in=np.ascontiguousarray(h0t).astype(BF16)))

    res = run_bass_kernel_spmd(nc, in_maps, core_ids=list(range(NCORES)))

    # decode hs: [T/HS, 128, HS, NK, BL] -> h[t, s, kc*128+p]
    h_dec = []
    for c in range(NCORES):
        a = res.results[c]["hs"].reshape(T // HS_BLOCK, 128, HS_BLOCK, NK, BL)
        a = a.transpose(0, 2, 4, 3, 1).reshape(T, BL, D).astype(F32)
        h_dec.append(a)                                      # [T, BL, D]

    hf = np.concatenate([h_dec[c] for c in range(4)], axis=1)      # [T,32,D]
    hbr = np.concatenate([h_dec[c] for c in range(4, 8)], axis=1)
    hf = hf.transpose(1, 0, 2)                                     # [B,T,D]
    hb = _seq_flip(hbr.transpose(1, 0, 2), lengths)
    feats = np.concatenate([hf, hb], axis=-1)                      # [B,T,2D]
    emissions = feats @ np.asarray(W_emit, F32).T + np.asarray(b_emit, F32)

    e = emissions.astype(np.float64)
    tr = np.asarray(trans, np.float64)
    st = np.asarray(start_trans, np.float64)
    et = np.asarray(end_trans, np.float64)
    mask = np.arange(T)[None, :] < lengths[:, None]
    alpha = e[:, 0] + st
    expTrT = np.exp(tr).T
    for t in range(1, T):
        m = alpha.max(axis=1, keepdims=True)
        new = e[:, t] + m + np.log(np.exp(alpha - m) @ expTrT)
        alpha = np.where(mask[:, t][:, None], new, alpha)
    fwd = _logsumexp(alpha + et, axis=-1)
    e_tag = np.take_along_axis(e, tags[..., None], axis=-1)[..., 0]
    step_scores = tr[tags[:, 1:], tags[:, :-1]] + e_tag[:, 1:]
    last_tag = np.take_along_axis(tags, (lengths - 1)[:, None], axis=1)[:, 0]
    gold = (st[tags[:, 0]] + e_tag[:, 0]
            + np.sum(np.where(mask[:, 1:], step_scores, 0.0), axis=-1)
            + et[last_tag])
    return np.float32(np.sum(fwd - gold))


# revision 21
# speedup vs baseline: 4.7786x; 1.0036x over previous
"""BiLSTM-CRF loss on 8 Trainium2 NeuronCores.

Strategy (v3, fused single kernel):
  - Direction-split: cores 0-3 forward LSTM, cores 4-7 backward LSTM (on
    host-pre-flipped input); batch (32) sharded 4 ways -> 8 sequences/core.
  - Transposed cell layout: gate dim on partitions, (chunk, seq) in the free
    dim. Recurrent + input-projection matmuls all accumulate into one psum
    tile per step (input projection is dependency-free and fills tensor-engine
    idle time, so there is no separate projection kernel and no xp roundtrip).
  - All-tanh gates: i/f/o rows of the weights are pre-scaled by 0.5 so
    sigmoid(x) = 0.5*(tanh(x/2)+1); state is kept as H2=2h (bf16) and C2=2c
    (f32), making the elementwise cell update exact with three fused
    scalar_tensor_tensor ops on gpsimd:
       A = (t_i+1)*t_g ; B = (t_f+1)*C2 ; C2' = 0.5*B + A ; tc = tanh(0.5*C2')
       H2' = (t_o+1)*tc
  - Host (numpy, fp64): embedding gather, sequence flips, emissions, CRF
    forward/gold score.
"""
import sys
import numpy as np

sys.path.insert(0, '/opt/trn_rl_repo')

import concourse.bacc as bacc
import concourse.mybir as mybir
from concourse.tile import TileContext
from concourse.bass_utils import run_bass_kernel_spmd
import ml_dtypes

BF16 = ml_dtypes.bfloat16
F32 = np.float32

B, T = 32, 512
V, D, L = 50257, 512, 48
NCORES = 8
BL = 8          # sequences per core
NM, NK = 16, 4  # gate chunks (128 each), h chunks (128 each)
W = BL * NM     # 128
HC = BL * NK    # 32
CH = T          # steps per kernel call (single call)
HS_BLOCK = 16

_TANH = mybir.ActivationFunctionType.Tanh
_SIG = mybir.ActivationFunctionType.Sigmoid
_ADD = mybir.AluOpType.add
_MULT = mybir.AluOpType.mult

_cache = {}


def _build(ch):
    nc = bacc.Bacc()
    dt = mybir.dt
    embT = nc.declare_dram_parameter("embT", [128, NK * ch * BL], dt.bfloat16,
                                     isOutput=False)
    whh = nc.declare_dram_parameter("whh", [128, NK * NM * 128], dt.bfloat16,
                                    isOutput=False)
    wih = nc.declare_dram_parameter("wih", [128, NK * NM * 128], dt.bfloat16,
                                    isOutput=False)
    biasb = nc.declare_dram_parameter("biasb", [128, W], dt.bfloat16,
                                      isOutput=False)
    ident = nc.declare_dram_parameter("ident", [128, 128], dt.bfloat16,
                                      isOutput=False)
    c_in = nc.declare_dram_parameter("c_in", [128, HC], dt.float32, isOutput=False)
    h_in = nc.declare_dram_parameter("h_in", [128, HC], dt.bfloat16, isOutput=False)
    hs = nc.declare_dram_parameter("hs", [ch // HS_BLOCK, 128, HS_BLOCK * HC],
                                   dt.bfloat16, isOutput=True)

    with TileContext(nc) as tc:
        with (
            tc.tile_pool(name="const", bufs=1) as cpool,
            tc.tile_pool(name="state", bufs=2) as spool,
            tc.tile_pool(name="t", bufs=2) as tpool,
            tc.tile_pool(name="ab", bufs=2) as abpool,
            tc.tile_pool(name="hsb", bufs=2) as hspool,
            tc.tile_pool(name="pg", bufs=2, space="PSUM") as pgpool,
        ):
            # balanced 3-way load: both HWDGE queues carry wih + 5/13 of whh
            # each; the Pool SWDGE queue carries the small tiles, the embT
            # heads, and the remaining 3/13 of whh, so all three queues drain
            # at roughly the same time (step 0 is gated on the last weight
            # byte)
            wih_sb = cpool.tile([128, NK * NM * 128], dt.bfloat16)
            whh_sb = cpool.tile([128, NK * NM * 128], dt.bfloat16)
            WTOT = NK * NM * 128
            s1 = (WTOT * 5) // 13
            nc.sync.dma_start(out=wih_sb[:, 0:WTOT // 2],
                              in_=wih[:, 0:WTOT // 2])
            nc.scalar.dma_start(out=wih_sb[:, WTOT // 2:],
                                in_=wih[:, WTOT // 2:])
            nc.sync.dma_start(out=whh_sb[:, 0:s1], in_=whh[:, 0:s1])
            nc.scalar.dma_start(out=whh_sb[:, s1:2 * s1], in_=whh[:, s1:2 * s1])
            bias_sb = cpool.tile([128, W], dt.bfloat16)
            nc.gpsimd.dma_start(out=bias_sb[:], in_=biasb[:])
            id_sb = cpool.tile([128, 128], dt.bfloat16)
            nc.gpsimd.dma_start(out=id_sb[:], in_=ident[:])
            c_prev = spool.tile([128, HC], dt.float32, tag="c")
            nc.gpsimd.dma_start(out=c_prev[:], in_=c_in[:])
            h_prev = spool.tile([128, HC], dt.bfloat16, tag="h")
            nc.gpsimd.dma_start(out=h_prev[:], in_=h_in[:])
            # embT: per-k-chunk head (first 64 steps) then tails, so step 0
            # isn't gated on the full 4 MB load
            embT_sb = cpool.tile([128, NK * ch * BL], dt.bfloat16)
            hd = min(64, ch) * BL
            for kc in range(NK):
                nc.gpsimd.dma_start(
                    out=embT_sb[:, kc * ch * BL:kc * ch * BL + hd],
                    in_=embT[:, kc * ch * BL:kc * ch * BL + hd])
            # Pool's share of whh rides after the heads
            nc.gpsimd.dma_start(out=whh_sb[:, 2 * s1:], in_=whh[:, 2 * s1:])
            # tails ride the SP queue behind the weights: Pool must stay free
            # for the per-step elementwise ops, Act for the activations
            for kc in range(NK):
                if ch * BL > hd:
                    nc.sync.dma_start(
                        out=embT_sb[:, kc * ch * BL + hd:(kc + 1) * ch * BL],
                        in_=embT[:, kc * ch * BL + hd:(kc + 1) * ch * BL])
            # dummy activation pre-loads the sigmoid/tanh table while the
            # weight DMAs are still in flight
            warm_sb = tpool.tile([1, 1], dt.float32, tag="warm")
            nc.scalar.activation(warm_sb[:], bias_sb[0:1, 0:1], _TANH)

            # gate chunk m (PyTorch order i0-3 f4-7 g8-11 o12-15) ->
            # (psum tile, col) — g gets its own tile and runs first so its
            # tanh can start while i/f/o matmuls still accumulate.
            def slot(pg_if, pg_g, pg_o, m):
                if m < 8:
                    return pg_if[:, m * BL:(m + 1) * BL]
                if m < 12:
                    return pg_g[:, (m - 8) * BL:(m - 7) * BL]
                return pg_o[:, (m - 12) * BL:(m - 11) * BL]

            MM_ORDER = [8, 9, 10, 11, 0, 1, 2, 3, 4, 5, 6, 7, 12, 13, 14, 15]
            hs_buf = None
            for j in range(ch):
                pg_if = pgpool.tile([128, 64], dt.float32, tag="pgif")
                pg_g = pgpool.tile([128, 32], dt.float32, tag="pgg")
                pg_o = pgpool.tile([128, 32], dt.float32, tag="pgo")
                nc.tensor.matmul(pg_g[:], id_sb[:], bias_sb[:, 64:96],
                                 start=True, stop=False, skip_group_check=True)
                nc.tensor.matmul(pg_if[:], id_sb[:], bias_sb[:, 0:64],
                                 start=True, stop=False, skip_group_check=True)
                nc.tensor.matmul(pg_o[:], id_sb[:], bias_sb[:, 96:128],
                                 start=True, stop=False, skip_group_check=True)
                for m in range(NM):
                    o = slot(pg_if, pg_g, pg_o, m)
                    for kc in range(NK):
                        nc.tensor.matmul(
                            o, wih_sb[:, (kc * NM + m) * 128:(kc * NM + m + 1) * 128],
                            embT_sb[:, (kc * ch + j) * BL:(kc * ch + j) * BL + BL],
                            start=False, stop=False, skip_group_check=True)
                # g-gate matmuls in kc waves (h arrives in two halves, so the
                # kc0/1 wave starts one Pool-op earlier); i/f/o follow
                for kcs, ms in (([0, 1], [8, 9, 10, 11]),
                                ([2, 3], [8, 9, 10, 11]),
                                (range(NK), [0, 1, 2, 3, 4, 5, 6, 7,
                                             12, 13, 14, 15])):
                    for m in ms:
                        o = slot(pg_if, pg_g, pg_o, m)
                        for kc in kcs:
                            nc.tensor.matmul(
                                o, whh_sb[:, (kc * NM + m) * 128:(kc * NM + m + 1) * 128],
                                h_prev[:, kc * BL:(kc + 1) * BL],
                                start=False, stop=(kc == NK - 1),
                                skip_group_check=True)
                tg_sb = tpool.tile([128, HC], dt.float32, tag="tg")
                nc.scalar.activation(tg_sb[:], pg_g[:], _TANH)
                sif_sb = tpool.tile([128, 64], dt.float32, tag="sif")
                nc.scalar.activation(sif_sb[:], pg_if[:], _SIG)
                so_sb = tpool.tile([128, HC], dt.float32, tag="so")
                nc.scalar.activation(so_sb[:], pg_o[:], _SIG)
                # plain gpsimd tensor ops, split in thirds so each slice of
                # c' = s_i*t_g + s_f*c retires as early as possible
                b_sb = abpool.tile([128, HC], dt.float32, tag="b", name=f"B_{j}")
                a_sb = abpool.tile([128, HC], dt.float32, tag="a", name=f"A_{j}")
                c_new = spool.tile([128, HC], dt.float32, tag="c", name=f"C_{j}")
                for lo, hi in ((0, 11), (11, 22), (22, HC)):
                    nc.gpsimd.tensor_mul(a_sb[:, lo:hi], sif_sb[:, lo:hi],
                                         tg_sb[:, lo:hi])
                    nc.gpsimd.tensor_mul(b_sb[:, lo:hi], sif_sb[:, 32 + lo:32 + hi],
                                         c_prev[:, lo:hi])
                for lo, hi in ((0, 11), (11, 22), (22, HC)):
                    nc.gpsimd.tensor_add(c_new[:, lo:hi], a_sb[:, lo:hi],
                                         b_sb[:, lo:hi])
                tc_sb = tpool.tile([128, HC], dt.float32, tag="tc", name=f"TC_{j}")
                nc.scalar.activation(tc_sb[:], c_new[:], _TANH)
                if j % HS_BLOCK == 0:
                    hs_buf = hspool.tile([128, HS_BLOCK * HC], dt.bfloat16,
                                         tag="hsb")
                base = (j % HS_BLOCK) * HC
                hh = HC // 2
                nc.gpsimd.tensor_mul(hs_buf[:, base:base + hh],
                                     so_sb[:, 0:hh], tc_sb[:, 0:hh])
                nc.gpsimd.tensor_mul(hs_buf[:, base + hh:base + HC],
                                     so_sb[:, hh:HC], tc_sb[:, hh:HC])
                h_new = hs_buf[:, base:base + HC]
                if j % HS_BLOCK == HS_BLOCK - 1:
                    nc.sync.dma_start(out=hs[j // HS_BLOCK], in_=hs_buf[:])
                c_prev, h_prev = c_new, h_new
    nc.finalize()
    return nc


def _pack_w(w):
    """[2048, 512] -> lhsT blocks [128, 64*128]; col (kc*16+m)*128+q =
    w[m*128+q, kc*128+p] at partition p."""
    w4 = np.asarray(w, F32).reshape(NM, 128, NK, 128)   # [m, q, kc, p]
    return np.ascontiguousarray(
        w4.transpose(3, 2, 0, 1).reshape(128, NK * NM * 128)).astype(BF16)


def _pack_x(x):
    """[BL, T, D] -> embT [128, NK*T*BL]; col (kc*T*BL + t*BL + s)."""
    a = np.asarray(x, F32).transpose(2, 1, 0)            # [D, T, BL]
    a = a.reshape(NK, 128, T * BL).transpose(1, 0, 2)    # [128, NK, T*BL]
    return np.ascontiguousarray(a.reshape(128, NK * T * BL)).astype(BF16)


def _seq_flip(x, lengths):
    t = np.arange(x.shape[1])[None, :]
    idx = lengths[:, None] - 1 - t
    idx = np.where(idx >= 0, idx, t)
    return np.take_along_axis(x, idx[:, :, None], axis=1)


def _logsumexp(a, axis):
    m = np.max(a, axis=axis, keepdims=True)
    return np.squeeze(m, axis) + np.log(np.sum(np.exp(a - m), axis=axis))


def kernel(tokens, tags, lengths, embed, W_ih_f, W_hh_f, b_ih_f, b_hh_f,
           W_ih_b, W_hh_b, b_ih_b, b_hh_b, init_hidden, W_emit, b_emit,
           start_trans, trans, end_trans):
    tokens = np.asarray(tokens).astype(np.int64)
    tags = np.asarray(tags).astype(np.int64)
    lengths = np.asarray(lengths).astype(np.int64)
    embed = np.asarray(embed, F32)

    if "rec" not in _cache:
        _cache["rec"] = _build(CH)
    nc = _cache["rec"]

    emb = embed[tokens]                      # [B,T,D] f32
    embr = _seq_flip(emb, lengths)           # reversed input for bwd lstm

    ident = np.eye(128, dtype=BF16)

    in_maps = []
    for c in range(NCORES):
        d = 0 if c < 4 else 1
        W_ih, W_hh = (W_ih_f, W_hh_f) if d == 0 else (W_ih_b, W_hh_b)
        b_sum = (np.asarray(b_ih_f, F32) + np.asarray(b_hh_f, F32)) if d == 0 \
            else (np.asarray(b_ih_b, F32) + np.asarray(b_hh_b, F32))
        wih_p = _pack_w(np.asarray(W_ih, F32))
        whh_p = _pack_w(np.asarray(W_hh, F32))
        be = b_sum.reshape(NM, 128).T                        # [p, m]
        biasb = np.ascontiguousarray(
            np.repeat(be[:, :, None], BL, axis=2).reshape(128, W)).astype(BF16)
        h0 = np.asarray(init_hidden, F32)[d]                 # [D]
        h0t = np.broadcast_to(h0.reshape(NK, 128).T[:, :, None],
                              (128, NK, BL)).reshape(128, HC)
        x = emb if d == 0 else embr
        sl = x[(c % 4) * BL:(c % 4 + 1) * BL]                # [BL, T, D]
        in_maps.append(dict(
            embT=_pack_x(sl), whh=whh_p, wih=wih_p, biasb=biasb, ident=ident,
            c_in=np.ascontiguousarray(h0t).astype(F32),
            h_in=np.ascontiguousarray(h0t).astype(BF16)))

    res = run_bass_kernel_spmd(nc, in_maps, core_ids=list(range(NCORES)))

    # decode hs: [T/HS, 128, HS, NK, BL] -> h[t, s, kc*128+p]
    h_dec = []
    for c in range(NCORES):
        a = res.results[c]["hs"].reshape(T // HS_BLOCK, 128, HS_BLOCK, NK, BL)
        a = a.transpose(0, 2, 4, 3, 1).reshape(T, BL, D).astype(F32)
        h_dec.append(a)                                      # [T, BL, D]

    hf = np.concatenate([h_dec[c] for c in range(4)], axis=1)      # [T,32,D]
    hbr = np.concatenate([h_dec[c] for c in range(4, 8)], axis=1)
    hf = hf.transpose(1, 0, 2)                                     # [B,T,D]
    hb = _seq_flip(hbr.transpose(1, 0, 2), lengths)
    feats = np.concatenate([hf, hb], axis=-1)                      # [B,T,2D]
    emissions = feats @ np.asarray(W_emit, F32).T + np.asarray(b_emit, F32)

    e = emissions.astype(np.float64)
    tr = np.asarray(trans, np.float64)
    st = np.asarray(start_trans, np.float64)
    et = np.asarray(end_trans, np.float64)
    mask = np.arange(T)[None, :] < lengths[:, None]
    alpha = e[:, 0] + st
    expTrT = np.exp(tr).T
    for t in range(1, T):
        m = alpha.max(axis=1, keepdims=True)
        new = e[:, t] + m + np.log(np.exp(alpha - m) @ expTrT)
        alpha = np.where(mask[:, t][:, None], new, alpha)
    fwd = _logsumexp(alpha + et, axis=-1)
    e_tag = np.take_along_axis(e, tags[..., None], axis=-1)[..., 0]
    step_scores = tr[tags[:, 1:], tags[:, :-1]] + e_tag[:, 1:]
    last_tag = np.take_along_axis(tags, (lengths - 1)[:, None], axis=1)[:, 0]
    gold = (st[tags[:, 0]] + e_tag[:, 0]
            + np.sum(np.where(mask[:, 1:], step_scores, 0.0), axis=-1)
            + et[last_tag])
    return np.float32(np.sum(fwd - gold))
